# revision 1
# baseline (speedup 1.0000x reference)
"""AdaptiveGNN (GCN+GAT+SAGE mixture) on 8 Trainium2 NeuronCores.

Strategy: destination-sharded graph parallelism.
 - Nodes split into 8 contiguous shards (6250 each). Core k computes every
   per-node output row for shard k.
 - Edges (with self-loops where the op needs them) are sorted by destination
   on the host and padded into a static per-window tile schedule shared by
   all 8 cores (window = 128 destination rows -> one PSUM accumulation).
 - Per edge-tile: indirect-DMA gather of source-node feature rows, a one-hot
   "selection" matrix built from the window-local destination ids (weighted
   by the per-edge coefficient: GCN norm / SAGE 1/deg / GAT exp(logit)),
   and a TensorE matmul that performs the segment-sum into PSUM.
 - Three sequential NEFF launches; the host only slices/concatenates shard
   arrays between launches (no FLOPs on host beyond index/graph prep and
   static weight folding).
   A: per-shard GAT attention projections (a_src/a_dst) + column sums of x.
   B: layer 1 of all three branches + gate MLP (outputs h1, hs1, and the
      pre-transformed GAT hidden h2 = elu(gat1)@W2 with its attn scalars).
   C: layer 2 of all three branches + gated mix -> final output rows.
"""

import sys

sys.path.insert(0, "/opt/trn_rl_repo")

import numpy as np

from concourse import bacc, bass, mybir, tile
from concourse.bass_utils import run_bass_kernel_spmd
import concourse.tile_sem_assignment as _tsa

# Clamp Tile's DMA-completion semaphore lanes to one HWDGE + one SWDGE lane.
# The kernel-tail Drain waits on every producer semaphore, and walrus'
# codegen rejects instructions with too many sync waits; fewer lanes keeps
# the wait list within the ISA limit (DMAs still spread over all 16 SDMA
# engines; only completion bookkeeping is serialized).
_tsa.NUM_HWDGE_SEMS = 8
_tsa.NUM_SWDGE_GLOBAL_SEMS = 8

F32 = mybir.dt.float32
I32 = mybir.dt.int32
AF = mybir.ActivationFunctionType
ALU = mybir.AluOpType

NC_N = 8          # cores
D = 64            # feature dim
H1 = 4            # GAT hidden heads
KB = 1            # edge tiles per gather batch (HW: 1 idx/partition)
NEG_SLOPE = 0.2
BN_EPS = 1e-5


# ----------------------------------------------------------------- host prep
def build_schedule(edge_index, n_nodes):
    """Sort edges (plus self-loops) by destination, shard by destination,
    and produce a tile schedule common to all cores plus per-core streams."""
    shard = n_nodes // NC_N
    nw = (shard + 127) // 128
    row = edge_index[0].astype(np.int64)
    col = edge_index[1].astype(np.int64)
    n_e = row.shape[0]
    loops = np.arange(n_nodes, dtype=np.int64)
    r_all = np.concatenate([row, loops])
    c_all = np.concatenate([col, loops])

    # GCN symmetric normalization (self-loops included)
    deg = np.bincount(c_all, minlength=n_nodes).astype(np.float64)
    dis = np.where(deg > 0, deg ** -0.5, 0.0)
    wnorm_all = (dis[r_all] * dis[c_all]).astype(np.float32)
    # SAGE mean weights (real edges only; zero on appended self-loops)
    cnt = np.bincount(col, minlength=n_nodes).astype(np.float64)
    ws = (1.0 / np.maximum(cnt, 1.0))[col].astype(np.float32)
    wsage_all = np.concatenate([ws, np.zeros(n_nodes, np.float32)])

    per_core = []
    counts = np.zeros((NC_N, nw), dtype=np.int64)
    for k in range(NC_N):
        lo, hi = k * shard, (k + 1) * shard
        sel = np.nonzero((c_all >= lo) & (c_all < hi))[0]
        cl = c_all[sel] - lo
        order = np.argsort(cl, kind="stable")
        sel = sel[order]
        cl = cl[order]
        w_of = cl // 128
        cnts = np.bincount(w_of, minlength=nw)
        counts[k] = cnts
        per_core.append((sel, cl, cnts))

    tiles_w = np.maximum(1, (counts.max(axis=0) + 127) // 128)  # tiles per window
    # round tile count so every gather batch of KB tiles stays simple
    T = int(tiles_w.sum())
    Tpad = ((T + KB - 1) // KB) * KB

    streams = []
    for k in range(NC_N):
        sel, cl, cnts = per_core[k]
        idx_row = np.zeros(Tpad * 128, np.int32)
        idx_col = np.zeros(Tpad * 128, np.int32)
        colrel = np.full(Tpad * 128, -1.0, np.float32)
        wnorm = np.zeros(Tpad * 128, np.float32)
        wsage = np.zeros(Tpad * 128, np.float32)
        pos = 0      # position in padded stream
        epos = 0     # position in this core's sorted edge list
        for w in range(nw):
            cw = int(cnts[w])
            seg = sel[epos:epos + cw]
            base = pos
            idx_row[base:base + cw] = r_all[seg]
            idx_col[base:base + cw] = c_all[seg]
            colrel[base:base + cw] = (cl[epos:epos + cw] % 128).astype(np.float32)
            wnorm[base:base + cw] = wnorm_all[seg]
            wsage[base:base + cw] = wsage_all[seg]
            epos += cw
            pos += int(tiles_w[w]) * 128
        st = {
            "idx_row": idx_row.reshape(Tpad, 128).T.copy(),
            "idx_col": idx_col.reshape(Tpad, 128).T.copy(),
            "colrel": colrel.reshape(Tpad, 128).T.copy(),
            "wnorm": wnorm.reshape(Tpad, 128).T.copy(),
            "wsage": wsage.reshape(Tpad, 128).T.copy(),
        }
        streams.append(st)
    return streams, [int(t) for t in tiles_w], Tpad, shard, nw


# ------------------------------------------------------------- common pieces
def _setup_consts(nc, tc, pools, dr):
    # NB: every tile that can feed a TensorE matmul is produced by DVE so the
    # matmul carries at most one semaphore wait (walrus limit on Matmult).
    # identity/iota come in as host constants so gpsimd stays silent (its
    # engine semaphore would otherwise show up in the kernel-tail drain).
    const = pools["const"]
    ident = _load_w(nc, const, dr["cident"], (128, 128), "ident")
    iota_f = _load_w(nc, const, dr["ciota"], (128, 128), "iota_f")
    ones_col = const.tile([128, 1], F32, tag="ones_col")
    nc.vector.memset(ones_col[:], 1.0)
    ones_row = const.tile([1, 128], F32, tag="ones_row")
    nc.vector.memset(ones_row[:], 1.0)
    return ident, iota_f, ones_col, ones_row


def _load_w(nc, pool, dram, shape, tag):
    ld = pool.tile(list(shape), F32, tag=tag + "_ld")
    nc.sync.dma_start(out=ld[:], in_=dram[:])
    t = pool.tile(list(shape), F32, tag=tag)
    nc.vector.tensor_copy(t[:], ld[:])
    return t


def _stage_out_dma(nc, st_tile, dram, nw, width):
    # staging [128, nw*width] -> dram [nw*128, width]
    out_ap = bass.AP(dram, 0, [[width, 128], [128 * width, nw], [1, width]])
    nc.sync.dma_start(out=out_ap, in_=st_tile[:].rearrange("p (w c) -> p w c", w=nw))


# ------------------------------------------------------------------ launch A
def build_A(n_nodes, shard, nw):
    npad = nw * 128
    nc = bacc.Bacc()
    xs = nc.dram_tensor("xs", [npad, D], F32, kind="ExternalInput")
    vcat = nc.dram_tensor("vcat", [D, 2 * H1], F32, kind="ExternalInput")
    drc = {"cident": nc.dram_tensor("cident", [128, 128], F32,
                                    kind="ExternalInput"),
           "ciota": nc.dram_tensor("ciota", [128, 128], F32,
                                   kind="ExternalInput")}
    a1 = nc.dram_tensor("a1", [npad, 2 * H1], F32, kind="ExternalOutput")
    csum = nc.dram_tensor("csum", [1, D], F32, kind="ExternalOutput")

    with tile.TileContext(nc) as tc:
        with (
            tc.tile_pool(name="const", bufs=1) as const,
            tc.tile_pool(name="sb", bufs=3) as sb,
            tc.tile_pool(name="ps", bufs=2, space="PSUM") as ps,
            tc.tile_pool(name="pcs", bufs=1, space="PSUM") as pcs,
        ):
            ident, iota_f, ones_col, ones_row = _setup_consts(
                nc, tc, {"const": const}, drc)
            vc = _load_w(nc, sb, vcat, (D, 2 * H1), "vc")
            csum_p = pcs.tile([1, D], F32, tag="csum")
            for w in range(nw):
                xt0 = sb.tile([128, D], F32, tag="xt0")
                nc.sync.dma_start(out=xt0[:], in_=xs[w * 128:(w + 1) * 128, :])
                xt = sb.tile([128, D], F32, tag="xt")
                nc.vector.tensor_copy(xt[:], xt0[:])
                pT = ps.tile([D, 128], F32, tag="pT")
                nc.tensor.matmul(out=pT[:], lhsT=xt[:], rhs=ident[:],
                                 is_transpose=True)
                xT = sb.tile([D, 128], F32, tag="xT")
                nc.vector.tensor_copy(xT[:], pT[:])
                pa = ps.tile([2 * H1, 128], F32, tag="pa")
                nc.tensor.matmul(out=pa[:], lhsT=vc[:], rhs=xT[:])
                aT = sb.tile([2 * H1, 128], F32, tag="aT")
                nc.vector.tensor_copy(aT[:], pa[:])
                pb = ps.tile([128, 2 * H1], F32, tag="pb")
                nc.tensor.matmul(out=pb[:], lhsT=aT[:],
                                 rhs=ident[:2 * H1, :2 * H1], is_transpose=True)
                ab = sb.tile([128, 2 * H1], F32, tag="ab")
                nc.vector.tensor_copy(ab[:], pb[:])
                nc.sync.dma_start(out=a1[w * 128:(w + 1) * 128, :], in_=ab[:])
                nc.tensor.matmul(out=csum_p[:], lhsT=ones_col[:], rhs=xt[:],
                                 start=(w == 0), stop=(w == nw - 1))
            cs = sb.tile([1, D], F32, tag="cs")
            nc.vector.tensor_copy(cs[:], csum_p[:])
            nc.sync.dma_start(out=csum[:], in_=cs[:])
    return nc


# ------------------------------------------------------------------ launch B
def build_B(n_nodes, shard, nw, tiles_w, Tpad):
    npad = nw * 128
    nc = bacc.Bacc()
    dr = {}
    for nm, shp, dt in [
        ("tx", [n_nodes, D + 1], F32),
        ("xs_pad", [npad, D], F32),
        ("sa", [128, Tpad * H1], F32), ("sb", [128, Tpad * H1], F32),
        ("idx_row", [128, Tpad], I32), ("idx_col", [128, Tpad], I32),
        ("colrel", [128, Tpad], F32), ("wnorm", [128, Tpad], F32),
        ("wsage", [128, Tpad], F32),
        ("csums", [NC_N, D], F32),
        ("gw1", [D, D], F32), ("gb1", [1, D], F32),
        ("gw2", [D, 3], F32), ("gb2", [1, 3], F32),
        ("gcn_w1", [D, D], F32), ("gcn1_s", [D, 1], F32), ("gcn1_b", [D, 1], F32),
        ("sage_wl1", [D, D], F32), ("sage_wr1", [D, D], F32),
        ("sage_bl1", [D, 1], F32),
        ("w2A", [128, D], F32), ("w2B", [128, D], F32),
        ("v2u2", [128, 4], F32), ("w1h", [D, 4 * D], F32),
        ("b1c", [128, 2], F32),
        ("cident", [128, 128], F32), ("ciota", [128, 128], F32),
    ]:
        dr[nm] = nc.dram_tensor(nm, shp, dt, kind="ExternalInput")
    out_h1 = nc.dram_tensor("h1", [npad, D], F32, kind="ExternalOutput")
    out_hs = nc.dram_tensor("hs1", [npad, D], F32, kind="ExternalOutput")
    out_h2 = nc.dram_tensor("h2", [npad, D], F32, kind="ExternalOutput")
    out_a2 = nc.dram_tensor("a2", [npad, 2], F32, kind="ExternalOutput")
    out_gate = nc.dram_tensor("gate", [1, 3], F32, kind="ExternalOutput")
    CW = D + 1       # gather row width (x | 1)

    with tile.TileContext(nc) as tc:
        with (
            tc.tile_pool(name="const", bufs=1) as const,
            tc.tile_pool(name="wts", bufs=1) as wts,
            tc.tile_pool(name="stream", bufs=1) as stream,
            tc.tile_pool(name="stage", bufs=1) as stage,
            tc.tile_pool(name="gat", bufs=8) as gat,
            tc.tile_pool(name="m", bufs=8) as mpool,
            tc.tile_pool(name="sm", bufs=3) as sm,
            tc.tile_pool(name="tl", bufs=4) as tl,
            tc.tile_pool(name="pacc", bufs=1, space="PSUM") as pacc,
            tc.tile_pool(name="ptmp", bufs=2, space="PSUM") as ptmp,
        ):
            ident, iota_f, ones_col, ones_row = _setup_consts(
                nc, tc, {"const": const}, dr)
            # ---- weights to SBUF
            W = {}
            for nm, shp in [
                ("gw1", (D, D)), ("gb1", (1, D)), ("gw2", (D, 3)), ("gb2", (1, 3)),
                ("gcn_w1", (D, D)), ("gcn1_s", (D, 1)), ("gcn1_b", (D, 1)),
                ("sage_wl1", (D, D)), ("sage_wr1", (D, D)), ("sage_bl1", (D, 1)),
                ("w2A", (128, D)), ("w2B", (128, D)), ("v2u2", (128, 4)),
                ("w1h", (D, 4 * D)), ("b1c", (128, 2)),
                ("csums", (NC_N, D)),
            ]:
                W[nm] = _load_w(nc, wts, dr[nm], shp, nm)

            # ---- gate MLP
            pxb = ptmp.tile([1, D], F32, tag="pt")
            nc.tensor.matmul(out=pxb[:], lhsT=ones_col[:NC_N, :1],
                             rhs=W["csums"][:])
            xbar = sm.tile([1, D], F32, tag="g_xbar")
            nc.vector.tensor_scalar(out=xbar[:], in0=pxb[:],
                                    scalar1=1.0 / n_nodes, scalar2=None,
                                    op0=ALU.mult)
            pxT = ptmp.tile([D, 1], F32, tag="pt")
            nc.tensor.matmul(out=pxT[:], lhsT=xbar[:], rhs=ident[:1, :1],
                             is_transpose=True)
            xbT = sm.tile([D, 1], F32, tag="g_xbT")
            nc.vector.tensor_copy(xbT[:], pxT[:])
            pg1 = ptmp.tile([1, D], F32, tag="pt")
            nc.tensor.matmul(out=pg1[:], lhsT=xbT[:], rhs=W["gw1"][:])
            g1 = sm.tile([1, D], F32, tag="g_g1")
            nc.vector.tensor_tensor(out=g1[:], in0=pg1[:], in1=W["gb1"][:],
                                    op=ALU.add)
            g1r = sm.tile([1, D], F32, tag="g_g1r")
            nc.vector.tensor_scalar(out=g1r[:], in0=g1[:], scalar1=0.0,
                                    scalar2=None, op0=ALU.max)
            pg1T = ptmp.tile([D, 1], F32, tag="pt")
            nc.tensor.matmul(out=pg1T[:], lhsT=g1r[:], rhs=ident[:1, :1],
                             is_transpose=True)
            g1T = sm.tile([D, 1], F32, tag="g_g1T")
            nc.vector.tensor_copy(g1T[:], pg1T[:])
            pg2 = ptmp.tile([1, 3], F32, tag="pt")
            nc.tensor.matmul(out=pg2[:], lhsT=g1T[:], rhs=W["gw2"][:])
            g2 = sm.tile([1, 3], F32, tag="g_g2")
            nc.vector.tensor_tensor(out=g2[:], in0=pg2[:], in1=W["gb2"][:],
                                    op=ALU.add)
            g2e = sm.tile([1, 3], F32, tag="g_g2e")
            nc.scalar.activation(out=g2e[:], in_=g2[:], func=AF.Exp)
            g2s = sm.tile([1, 1], F32, tag="g_g2s")
            nc.vector.tensor_reduce(out=g2s[:], in_=g2e[:],
                                    axis=mybir.AxisListType.X, op=ALU.add)
            g2r = sm.tile([1, 1], F32, tag="g_g2r")
            nc.vector.reciprocal(g2r[:], g2s[:])
            gate_sb = sm.tile([1, 3], F32, tag="g_gate")
            nc.vector.tensor_scalar(out=gate_sb[:], in0=g2e[:],
                                    scalar1=g2r[:, :1], scalar2=None,
                                    op0=ALU.mult)
            nc.sync.dma_start(out=out_gate[:], in_=gate_sb[:])

            # ---- streams to SBUF
            S = {}
            for nm, dt in [("idx_row", I32), ("idx_col", I32), ("colrel", F32),
                           ("wnorm", F32), ("wsage", F32)]:
                t = stream.tile([128, Tpad], dt, tag=nm)
                nc.sync.dma_start(out=t[:], in_=dr[nm][:])
                S[nm] = t
            for nm in ("sa", "sb"):
                t = stream.tile([128, Tpad * H1], F32, tag=nm)
                nc.sync.dma_start(out=t[:], in_=dr[nm][:])
                S[nm] = t

            # ---- staging buffers
            st_h1 = stage.tile([128, nw * D], F32, tag="st_h1")
            st_hs = stage.tile([128, nw * D], F32, tag="st_hs")
            st_h2 = stage.tile([128, nw * D], F32, tag="st_h2")
            st_a2 = stage.tile([128, nw * 2], F32, tag="st_a2")

            # ---- edge loop
            n_groups = Tpad // KB
            Gs, Es = [None] * n_groups, [None] * n_groups

            def ensure_group(g):
                if Gs[g] is not None:
                    return
                Gt = gat.tile([128, KB * CW], F32, tag="G")
                nc.gpsimd.indirect_dma_start(
                    out=Gt[:], out_offset=None, in_=dr["tx"][:],
                    in_offset=bass.IndirectOffsetOnAxis(
                        ap=S["idx_row"][:, g * KB:(g + 1) * KB], axis=0))
                Gc = gat.tile([128, KB * CW], F32, tag="Gc")
                nc.vector.tensor_copy(Gc[:], Gt[:])
                zt = gat.tile([128, KB * H1], F32, tag="z")
                nc.vector.tensor_tensor(
                    out=zt[:],
                    in0=S["sa"][:, g * KB * H1:(g + 1) * KB * H1],
                    in1=S["sb"][:, g * KB * H1:(g + 1) * KB * H1],
                    op=ALU.add)
                zs = gat.tile([128, KB * H1], F32, tag="zs")
                nc.vector.tensor_scalar(out=zs[:], in0=zt[:],
                                        scalar1=NEG_SLOPE, scalar2=None,
                                        op0=ALU.mult)
                nc.vector.tensor_tensor(out=zt[:], in0=zt[:], in1=zs[:],
                                        op=ALU.max)
                et = gat.tile([128, KB * H1], F32, tag="E")
                nc.scalar.activation(out=et[:], in_=zt[:], func=AF.Exp)
                Gs[g], Es[g] = Gc, et

            t_glob = 0
            for w in range(nw):
                ntw = tiles_w[w]
                p_gcnT = pacc.tile([D, 128], F32, tag="p_gcnT")
                p_sageT = pacc.tile([D, 128], F32, tag="p_sageT")
                p_gath = []
                for h in range(H1):
                    pg = pacc.tile([128, D + 1], F32, tag=f"p_gat{h}")
                    p_gath.append(pg)
                for t in range(ntw):
                    g, j = divmod(t_glob, KB)
                    ensure_group(g)
                    Gt, et = Gs[g], Es[g]
                    g3 = Gt[:].rearrange("p (t c) -> p t c", t=KB)
                    g64 = g3[:, j, 0:D]
                    g65 = g3[:, j, 0:D + 1]
                    cr = S["colrel"][:, t_glob:t_glob + 1]
                    st, sp = (t == 0), (t == ntw - 1)
                    Mg = mpool.tile([128, 128], F32, tag="Mg")
                    nc.vector.tensor_scalar(
                        out=Mg[:], in0=iota_f[:], scalar1=cr,
                        scalar2=S["wnorm"][:, t_glob:t_glob + 1],
                        op0=ALU.is_equal, op1=ALU.mult)
                    nc.tensor.matmul(out=p_gcnT[:], lhsT=g64, rhs=Mg[:],
                                     start=st, stop=sp)
                    Ms = mpool.tile([128, 128], F32, tag="Ms")
                    nc.vector.tensor_scalar(
                        out=Ms[:], in0=iota_f[:], scalar1=cr,
                        scalar2=S["wsage"][:, t_glob:t_glob + 1],
                        op0=ALU.is_equal, op1=ALU.mult)
                    nc.tensor.matmul(out=p_sageT[:], lhsT=g64, rhs=Ms[:],
                                     start=st, stop=sp)
                    for h in range(H1):
                        Mh = mpool.tile([128, 128], F32, tag="Mh")
                        nc.vector.tensor_scalar(
                            out=Mh[:], in0=iota_f[:], scalar1=cr,
                            scalar2=et[:, H1 * j + h:H1 * j + h + 1],
                            op0=ALU.is_equal, op1=ALU.mult)
                        nc.tensor.matmul(
                            out=p_gath[h][:], lhsT=Mh[:], rhs=g65,
                            start=st, stop=sp)
                    t_glob += 1

                # ---------- window tails ----------
                # GCN1: h1 = relu(s*(W1^T aggT) + b) (feature-major layout)
                aggT = tl.tile([D, 128], F32, tag="aggT")
                nc.vector.tensor_copy(aggT[:], p_gcnT[:])
                ph1T = ptmp.tile([D, 128], F32, tag="pt")
                nc.tensor.matmul(out=ph1T[:], lhsT=W["gcn_w1"][:], rhs=aggT[:])
                h1Ts = tl.tile([D, 128], F32, tag="h1Ts")
                nc.scalar.activation(out=h1Ts[:], in_=ph1T[:], func=AF.Relu,
                                     scale=W["gcn1_s"][:, :1],
                                     bias=W["gcn1_b"][:, :1])
                h1Tv = tl.tile([D, 128], F32, tag="h1Tv")
                nc.vector.tensor_copy(h1Tv[:], h1Ts[:])
                ph1 = ptmp.tile([128, D], F32, tag="pt")
                nc.tensor.matmul(out=ph1[:], lhsT=h1Tv[:], rhs=ident[:D, :D],
                                 is_transpose=True)
                nc.vector.tensor_copy(st_h1[:, w * D:(w + 1) * D], ph1[:])

                # GAT1 heads: head_h = (sum exp*x)/den ; x2T_h = W_h^T head_h^T
                x2TA = tl.tile([128, 128], F32, tag="x2TA")
                x2TB = tl.tile([128, 128], F32, tag="x2TB")
                for h in range(H1):
                    rd = tl.tile([128, 1], F32, tag="rd")
                    nc.vector.reciprocal(rd[:], p_gath[h][:, D:D + 1])
                    hd_sb = tl.tile([128, D], F32, tag="hd_sb")
                    nc.vector.tensor_scalar(
                        out=hd_sb[:], in0=p_gath[h][:, 0:D],
                        scalar1=rd[:, :1], scalar2=None, op0=ALU.mult)
                    pht = ptmp.tile([D, 128], F32, tag="pt")
                    nc.tensor.matmul(out=pht[:], lhsT=hd_sb[:], rhs=ident[:],
                                     is_transpose=True)
                    hdT = tl.tile([D, 128], F32, tag="hdT_g")
                    nc.vector.tensor_copy(hdT[:], pht[:])
                    pxh = ptmp.tile([D, 128], F32, tag="pt")
                    nc.tensor.matmul(out=pxh[:],
                                     lhsT=W["w1h"][:, h * D:(h + 1) * D],
                                     rhs=hdT[:])
                    stgt = x2TA if h < 2 else x2TB
                    nc.vector.tensor_copy(
                        stgt[(h % 2) * D:(h % 2 + 1) * D, :], pxh[:])
                x2T = []
                for half, px in enumerate((x2TA, x2TB)):
                    yT = tl.tile([128, 128], F32, tag="yT")
                    nc.vector.tensor_scalar(
                        out=yT[:], in0=px[:],
                        scalar1=W["b1c"][:, half:half + 1], scalar2=None,
                        op0=ALU.add)
                    ymin = tl.tile([128, 128], F32, tag="ymin")
                    nc.vector.tensor_scalar(out=ymin[:], in0=yT[:],
                                            scalar1=0.0, scalar2=None,
                                            op0=ALU.min)
                    yexp = tl.tile([128, 128], F32, tag="yexp")
                    nc.scalar.activation(out=yexp[:], in_=ymin[:], func=AF.Exp)
                    ye1 = tl.tile([128, 128], F32, tag="ye1")
                    nc.vector.tensor_scalar(out=ye1[:], in0=yexp[:],
                                            scalar1=-1.0, scalar2=None,
                                            op0=ALU.add)
                    ymax = tl.tile([128, 128], F32, tag="ymax")
                    nc.vector.tensor_scalar(out=ymax[:], in0=yT[:],
                                            scalar1=0.0, scalar2=None,
                                            op0=ALU.max)
                    xt = tl.tile([128, 128], F32, tag=f"x2T{half}")
                    nc.vector.tensor_tensor(out=xt[:], in0=ymax[:],
                                            in1=ye1[:], op=ALU.add)
                    x2T.append(xt)
                ph2T = ptmp.tile([D, 128], F32, tag="pt")
                nc.tensor.matmul(out=ph2T[:], lhsT=W["w2A"][:], rhs=x2T[0][:],
                                 start=True, stop=False)
                nc.tensor.matmul(out=ph2T[:], lhsT=W["w2B"][:], rhs=x2T[1][:],
                                 start=False, stop=True)
                pa2T = ptmp.tile([2, 128], F32, tag="pt")
                nc.tensor.matmul(out=pa2T[:], lhsT=W["v2u2"][:, 0:2],
                                 rhs=x2T[0][:], start=True, stop=False)
                nc.tensor.matmul(out=pa2T[:], lhsT=W["v2u2"][:, 2:4],
                                 rhs=x2T[1][:], start=False, stop=True)
                h2Ts = tl.tile([D, 128], F32, tag="h2Ts")
                nc.vector.tensor_copy(h2Ts[:], ph2T[:])
                ph2 = ptmp.tile([128, D], F32, tag="pt")
                nc.tensor.matmul(out=ph2[:], lhsT=h2Ts[:], rhs=ident[:D, :D],
                                 is_transpose=True)
                nc.vector.tensor_copy(st_h2[:, w * D:(w + 1) * D], ph2[:])
                a2Ts = tl.tile([2, 128], F32, tag="a2Ts")
                nc.vector.tensor_copy(a2Ts[:], pa2T[:])
                pa2 = ptmp.tile([128, 2], F32, tag="pt")
                nc.tensor.matmul(out=pa2[:], lhsT=a2Ts[:], rhs=ident[:2, :2],
                                 is_transpose=True)
                nc.vector.tensor_copy(st_a2[:, w * 2:(w + 1) * 2], pa2[:])

                # SAGE1
                meanT = tl.tile([D, 128], F32, tag="meanT")
                nc.vector.tensor_copy(meanT[:], p_sageT[:])
                xd0 = tl.tile([128, D], F32, tag="xd0")
                nc.sync.dma_start(out=xd0[:],
                                  in_=dr["xs_pad"][w * 128:(w + 1) * 128, :])
                xd = tl.tile([128, D], F32, tag="xd")
                nc.vector.tensor_copy(xd[:], xd0[:])
                pxdT = ptmp.tile([D, 128], F32, tag="pt")
                nc.tensor.matmul(out=pxdT[:], lhsT=xd[:], rhs=ident[:],
                                 is_transpose=True)
                xdT = tl.tile([D, 128], F32, tag="xdT")
                nc.vector.tensor_copy(xdT[:], pxdT[:])
                psT = ptmp.tile([D, 128], F32, tag="pt")
                nc.tensor.matmul(out=psT[:], lhsT=W["sage_wl1"][:],
                                 rhs=meanT[:], start=True, stop=False)
                nc.tensor.matmul(out=psT[:], lhsT=W["sage_wr1"][:],
                                 rhs=xdT[:], start=False, stop=True)
                sTs = tl.tile([D, 128], F32, tag="sTs")
                nc.scalar.activation(out=sTs[:], in_=psT[:], func=AF.Identity,
                                     bias=W["sage_bl1"][:, :1])
                sTv = tl.tile([D, 128], F32, tag="sTv")
                nc.vector.tensor_copy(sTv[:], sTs[:])
                ps_ = ptmp.tile([128, D], F32, tag="pt")
                nc.tensor.matmul(out=ps_[:], lhsT=sTv[:], rhs=ident[:D, :D],
                                 is_transpose=True)
                s_sb = tl.tile([128, D], F32, tag="s_sb")
                nc.vector.tensor_copy(s_sb[:], ps_[:])
                sq = tl.tile([128, D], F32, tag="sq")
                nc.vector.tensor_tensor(out=sq[:], in0=s_sb[:], in1=s_sb[:],
                                        op=ALU.mult)
                ssum = tl.tile([128, 1], F32, tag="ssum")
                nc.vector.tensor_reduce(out=ssum[:], in_=sq[:],
                                        axis=mybir.AxisListType.X, op=ALU.add)
                nc.vector.tensor_scalar(out=ssum[:], in0=ssum[:],
                                        scalar1=1e-24, scalar2=None,
                                        op0=ALU.add)
                rs = tl.tile([128, 1], F32, tag="rs")
                nc.vector.reciprocal(rs[:], ssum[:])
                rq = tl.tile([128, 1], F32, tag="rq")
                nc.scalar.activation(out=rq[:], in_=rs[:], func=AF.Sqrt)
                nc.vector.tensor_scalar(out=st_hs[:, w * D:(w + 1) * D],
                                        in0=s_sb[:], scalar1=rq[:, :1],
                                        scalar2=0.0, op0=ALU.mult,
                                        op1=ALU.max)
                # free the gather group tiles we're done with
                if w == nw - 1 or True:
                    pass

            _stage_out_dma(nc, st_h1, out_h1, nw, D)
            _stage_out_dma(nc, st_hs, out_hs, nw, D)
            _stage_out_dma(nc, st_h2, out_h2, nw, D)
            _stage_out_dma(nc, st_a2, out_a2, nw, 2)
    return nc


# ------------------------------------------------------------------ launch C
def build_C(n_nodes, shard, nw, tiles_w, Tpad):
    npad = nw * 128
    nc = bacc.Bacc()
    dr = {}
    for nm, shp, dt in [
        ("tcat", [n_nodes, 2 * D + 1 + D], F32),
        ("sa2", [128, Tpad], F32), ("sb2", [128, Tpad], F32),
        ("hs_pad", [npad, D], F32),
        ("idx_row", [128, Tpad], I32), ("idx_col", [128, Tpad], I32),
        ("colrel", [128, Tpad], F32), ("wnorm", [128, Tpad], F32),
        ("wsage", [128, Tpad], F32),
        ("gcn_w2", [D, D], F32), ("gcn_b2c", [D, 1], F32),
        ("sage_wl2", [D, D], F32), ("sage_wr2", [D, D], F32),
        ("sage_bl2c", [D, 1], F32), ("gat_b2r", [1, D], F32),
        ("gate", [1, 3], F32),
        ("cident", [128, 128], F32), ("ciota", [128, 128], F32),
    ]:
        dr[nm] = nc.dram_tensor(nm, shp, dt, kind="ExternalInput")
    out = nc.dram_tensor("out", [npad, D], F32, kind="ExternalOutput")

    with tile.TileContext(nc) as tc:
        with (
            tc.tile_pool(name="const", bufs=1) as const,
            tc.tile_pool(name="wts", bufs=1) as wts,
            tc.tile_pool(name="stream", bufs=1) as stream,
            tc.tile_pool(name="stage", bufs=1) as stage,
            tc.tile_pool(name="gat", bufs=8) as gat,
            tc.tile_pool(name="m", bufs=8) as mpool,
            tc.tile_pool(name="tl", bufs=4) as tl,
            tc.tile_pool(name="pacc", bufs=1, space="PSUM") as pacc,
            tc.tile_pool(name="ptmp", bufs=2, space="PSUM") as ptmp,
        ):
            ident, iota_f, ones_col, ones_row = _setup_consts(
                nc, tc, {"const": const}, dr)
            W = {}
            for nm, shp in [
                ("gcn_w2", (D, D)), ("gcn_b2c", (D, 1)),
                ("sage_wl2", (D, D)), ("sage_wr2", (D, D)),
                ("sage_bl2c", (D, 1)), ("gat_b2r", (1, D)), ("gate", (1, 3)),
            ]:
                W[nm] = _load_w(nc, wts, dr[nm], shp, nm)
            # gate scalar broadcasts
            pw128 = ptmp.tile([128, 3], F32, tag="pt")
            nc.tensor.matmul(out=pw128[:], lhsT=ones_row[:], rhs=W["gate"][:])
            wc = wts.tile([128, 3], F32, tag="wc")
            nc.vector.tensor_copy(wc[:], pw128[:])
            pw64 = ptmp.tile([D, 3], F32, tag="pt")
            nc.tensor.matmul(out=pw64[:], lhsT=ones_row[:1, :D],
                             rhs=W["gate"][:])
            w64 = wts.tile([D, 3], F32, tag="w64")
            nc.vector.tensor_copy(w64[:], pw64[:])
            # w0*gcn_b2, w2*sage_bl2 columns
            b2w0 = wts.tile([D, 1], F32, tag="b2w0")
            nc.vector.tensor_scalar(out=b2w0[:], in0=W["gcn_b2c"][:],
                                    scalar1=w64[:, 0:1], scalar2=None,
                                    op0=ALU.mult)
            # gat bias term: w1 * b2 broadcast [128, D]
            pbg = ptmp.tile([128, D], F32, tag="pt")
            nc.tensor.matmul(out=pbg[:], lhsT=ones_row[:], rhs=W["gat_b2r"][:])
            bgat = wts.tile([128, D], F32, tag="bgat")
            nc.vector.tensor_scalar(out=bgat[:], in0=pbg[:],
                                    scalar1=wc[:, 1:2], scalar2=None,
                                    op0=ALU.mult)

            S = {}
            for nm, dt in [("idx_row", I32), ("idx_col", I32), ("colrel", F32),
                           ("wnorm", F32), ("wsage", F32),
                           ("sa2", F32), ("sb2", F32)]:
                t = stream.tile([128, Tpad], dt, tag=nm)
                nc.sync.dma_start(out=t[:], in_=dr[nm][:])
                S[nm] = t

            st_out = stage.tile([128, nw * D], F32, tag="st_out")

            n_groups = Tpad // KB
            Gs1, Gs2, Gs3, Es = ([None] * n_groups for _ in range(4))

            TW = 2 * D + 1 + D         # [h1 | h2 | 1 | hs1]

            def ensure_group(g):
                if Gs1[g] is not None:
                    return
                sl = S["idx_row"][:, g * KB:(g + 1) * KB]
                G0 = gat.tile([128, KB * TW], F32, tag="G0")
                nc.gpsimd.indirect_dma_start(
                    out=G0[:], out_offset=None, in_=dr["tcat"][:],
                    in_offset=bass.IndirectOffsetOnAxis(ap=sl, axis=0))
                Gc = gat.tile([128, KB * TW], F32, tag="Gc")
                nc.vector.tensor_copy(Gc[:], G0[:])
                z2 = gat.tile([128, KB], F32, tag="z2")
                nc.vector.tensor_tensor(
                    out=z2[:],
                    in0=S["sa2"][:, g * KB:(g + 1) * KB],
                    in1=S["sb2"][:, g * KB:(g + 1) * KB],
                    op=ALU.add)
                z2s = gat.tile([128, KB], F32, tag="z2s")
                nc.vector.tensor_scalar(out=z2s[:], in0=z2[:],
                                        scalar1=NEG_SLOPE, scalar2=None,
                                        op0=ALU.mult)
                nc.vector.tensor_tensor(out=z2[:], in0=z2[:], in1=z2s[:],
                                        op=ALU.max)
                e2 = gat.tile([128, KB], F32, tag="E2")
                nc.scalar.activation(out=e2[:], in_=z2[:], func=AF.Exp)
                Gs1[g], Gs2[g], Gs3[g], Es[g] = Gc, Gc, Gc, e2

            t_glob = 0
            for w in range(nw):
                ntw = tiles_w[w]
                p_g2T = pacc.tile([D, 128], F32, tag="p_g2T")
                p_s2T = pacc.tile([D, 128], F32, tag="p_s2T")
                p_gat2 = pacc.tile([128, D + 1], F32, tag="p_gat2")
                for t in range(ntw):
                    g, j = divmod(t_glob, KB)
                    ensure_group(g)
                    gtile = Gs1[g][:].rearrange("p (t c) -> p t c", t=KB)
                    g1s = gtile[:, j, 0:D]
                    g2s = gtile[:, j, D:2 * D + 1]
                    g3s = gtile[:, j, 2 * D + 1:3 * D + 1]
                    cr = S["colrel"][:, t_glob:t_glob + 1]
                    st, sp = (t == 0), (t == ntw - 1)
                    Mg = mpool.tile([128, 128], F32, tag="Mg")
                    nc.vector.tensor_scalar(
                        out=Mg[:], in0=iota_f[:], scalar1=cr,
                        scalar2=S["wnorm"][:, t_glob:t_glob + 1],
                        op0=ALU.is_equal, op1=ALU.mult)
                    nc.tensor.matmul(out=p_g2T[:], lhsT=g1s, rhs=Mg[:],
                                     start=st, stop=sp)
                    Ms = mpool.tile([128, 128], F32, tag="Ms")
                    nc.vector.tensor_scalar(
                        out=Ms[:], in0=iota_f[:], scalar1=cr,
                        scalar2=S["wsage"][:, t_glob:t_glob + 1],
                        op0=ALU.is_equal, op1=ALU.mult)
                    nc.tensor.matmul(out=p_s2T[:], lhsT=g3s, rhs=Ms[:],
                                     start=st, stop=sp)
                    Mh = mpool.tile([128, 128], F32, tag="Mh")
                    nc.vector.tensor_scalar(
                        out=Mh[:], in0=iota_f[:], scalar1=cr,
                        scalar2=Es[g][:, j:j + 1],
                        op0=ALU.is_equal, op1=ALU.mult)
                    nc.tensor.matmul(out=p_gat2[:], lhsT=Mh[:], rhs=g2s,
                                     start=st, stop=sp)
                    t_glob += 1

                # GCN2 (+w0, +w0*b2)
                aggT = tl.tile([D, 128], F32, tag="aggT")
                nc.vector.tensor_copy(aggT[:], p_g2T[:])
                poT = ptmp.tile([D, 128], F32, tag="pt")
                nc.tensor.matmul(out=poT[:], lhsT=W["gcn_w2"][:], rhs=aggT[:])
                oTs = tl.tile([D, 128], F32, tag="oTs")
                nc.scalar.activation(out=oTs[:], in_=poT[:], func=AF.Identity,
                                     scale=w64[:, 0:1], bias=b2w0[:, :1])
                oTv = tl.tile([D, 128], F32, tag="oTv")
                nc.vector.tensor_copy(oTv[:], oTs[:])
                po = ptmp.tile([128, D], F32, tag="pt")
                nc.tensor.matmul(out=po[:], lhsT=oTv[:], rhs=ident[:D, :D],
                                 is_transpose=True)
                ogcn = tl.tile([128, D], F32, tag="ogcn")
                nc.vector.tensor_copy(ogcn[:], po[:])

                # GAT2 (+w1)
                rd = tl.tile([128, 1], F32, tag="rd")
                nc.vector.reciprocal(rd[:], p_gat2[:, D:D + 1])
                ogat = tl.tile([128, D], F32, tag="ogat")
                nc.vector.tensor_scalar(out=ogat[:], in0=p_gat2[:, 0:D],
                                        scalar1=rd[:, :1],
                                        scalar2=wc[:, 1:2],
                                        op0=ALU.mult, op1=ALU.mult)

                # SAGE2 (+w2)
                meanT = tl.tile([D, 128], F32, tag="meanT")
                nc.vector.tensor_copy(meanT[:], p_s2T[:])
                hd0 = tl.tile([128, D], F32, tag="hd0")
                nc.sync.dma_start(out=hd0[:],
                                  in_=dr["hs_pad"][w * 128:(w + 1) * 128, :])
                hd = tl.tile([128, D], F32, tag="hd")
                nc.vector.tensor_copy(hd[:], hd0[:])
                phdT = ptmp.tile([D, 128], F32, tag="pt")
                nc.tensor.matmul(out=phdT[:], lhsT=hd[:], rhs=ident[:],
                                 is_transpose=True)
                hdT = tl.tile([D, 128], F32, tag="hdT")
                nc.vector.tensor_copy(hdT[:], phdT[:])
                psT = ptmp.tile([D, 128], F32, tag="pt")
                nc.tensor.matmul(out=psT[:], lhsT=W["sage_wl2"][:],
                                 rhs=meanT[:], start=True, stop=False)
                nc.tensor.matmul(out=psT[:], lhsT=W["sage_wr2"][:],
                                 rhs=hdT[:], start=False, stop=True)
                sTs = tl.tile([D, 128], F32, tag="sTs")
                nc.scalar.activation(out=sTs[:], in_=psT[:], func=AF.Identity,
                                     bias=W["sage_bl2c"][:, :1])
                sTv = tl.tile([D, 128], F32, tag="sTv")
                nc.vector.tensor_copy(sTv[:], sTs[:])
                ps_ = ptmp.tile([128, D], F32, tag="pt")
                nc.tensor.matmul(out=ps_[:], lhsT=sTv[:], rhs=ident[:D, :D],
                                 is_transpose=True)
                s_sb = tl.tile([128, D], F32, tag="s_sb")
                nc.vector.tensor_copy(s_sb[:], ps_[:])
                sq = tl.tile([128, D], F32, tag="sq")
                nc.vector.tensor_tensor(out=sq[:], in0=s_sb[:], in1=s_sb[:],
                                        op=ALU.mult)
                ssum = tl.tile([128, 1], F32, tag="ssum")
                nc.vector.tensor_reduce(out=ssum[:], in_=sq[:],
                                        axis=mybir.AxisListType.X, op=ALU.add)
                nc.vector.tensor_scalar(out=ssum[:], in0=ssum[:],
                                        scalar1=1e-24, scalar2=None,
                                        op0=ALU.add)
                rs = tl.tile([128, 1], F32, tag="rs")
                nc.vector.reciprocal(rs[:], ssum[:])
                rq = tl.tile([128, 1], F32, tag="rq")
                nc.scalar.activation(out=rq[:], in_=rs[:], func=AF.Sqrt)
                osage = tl.tile([128, D], F32, tag="osage")
                nc.vector.tensor_scalar(out=osage[:], in0=s_sb[:],
                                        scalar1=rq[:, :1],
                                        scalar2=wc[:, 2:3],
                                        op0=ALU.mult, op1=ALU.mult)

                # mix
                mx1 = tl.tile([128, D], F32, tag="mx1")
                nc.vector.tensor_tensor(out=mx1[:], in0=ogcn[:], in1=ogat[:],
                                        op=ALU.add)
                mx2 = tl.tile([128, D], F32, tag="mx2")
                nc.vector.tensor_tensor(out=mx2[:], in0=mx1[:], in1=osage[:],
                                        op=ALU.add)
                nc.vector.tensor_tensor(out=st_out[:, w * D:(w + 1) * D],
                                        in0=mx2[:], in1=bgat[:], op=ALU.add)

            _stage_out_dma(nc, st_out, out, nw, D)
    return nc


# ---------------------------------------------------------------- host logic
DEBUG = {}
_PROG_CACHE = {}


def _run(nc, in_maps, trace=False):
    import time as _time
    if not nc.is_finalized():
        nc.finalize()   # Bacc.compile(): reg alloc + sync-wait legalization
    t0 = _time.perf_counter()
    res = run_bass_kernel_spmd(nc, in_maps, list(range(NC_N)), trace=trace)
    DEBUG.setdefault("run_walls", []).append(_time.perf_counter() - t0)
    if res.exec_time_ns:
        DEBUG.setdefault("exec_ns", []).append(res.exec_time_ns)
    return res.results


def gnn_forward(x, edge_index, gate_w1, gate_b1, gate_w2, gate_b2,
                gcn_w1, gcn_b1, bn_gamma, bn_beta, gcn_w2, gcn_b2,
                gat_w1, gat_att_src1, gat_att_dst1, gat_b1,
                gat_w2, gat_att_src2, gat_att_dst2, gat_b2,
                sage_wl1, sage_bl1, sage_wr1, sage_wl2, sage_bl2, sage_wr2,
                prebuilt=None):
    n_nodes = x.shape[0]
    x = np.asarray(x, np.float32)
    streams, tiles_w, Tpad, shard, nw = build_schedule(
        np.asarray(edge_index), n_nodes)
    npad = nw * 128

    # ---- host weight folding (weights only, no data)
    w1r = np.asarray(gat_w1, np.float32).reshape(D, H1, D)
    vsrc = np.einsum("chj,hj->ch", w1r, np.asarray(gat_att_src1, np.float32))
    vdst = np.einsum("chj,hj->ch", w1r, np.asarray(gat_att_dst1, np.float32))
    vcat = np.concatenate([vsrc, vdst], axis=1).astype(np.float32)  # [64,8]
    v2 = (np.asarray(gat_w2, np.float32) @
          np.asarray(gat_att_src2, np.float32)[0])  # [256]
    u2 = (np.asarray(gat_w2, np.float32) @
          np.asarray(gat_att_dst2, np.float32)[0])
    v2u2 = np.stack([v2[:128], u2[:128], v2[128:], u2[128:]],
                    axis=1).astype(np.float32)  # [128,4]
    bn_s = (np.asarray(bn_gamma, np.float32) /
            np.sqrt(np.float32(1.0 + BN_EPS)))
    gcn1_s = bn_s.reshape(D, 1).astype(np.float32)
    gcn1_b = (bn_s * np.asarray(gcn_b1, np.float32) +
              np.asarray(bn_beta, np.float32)).reshape(D, 1).astype(np.float32)

    def pad_shard(arr, k, width):
        out = np.zeros((npad, width), np.float32)
        out[:shard] = arr[k * shard:(k + 1) * shard]
        return out

    ck = (n_nodes, Tpad, tuple(tiles_w))
    if prebuilt is not None:
        nc_a, nc_b, nc_c = prebuilt
    elif ck in _PROG_CACHE:
        nc_a, nc_b, nc_c = _PROG_CACHE[ck]
    else:
        nc_a = build_A(n_nodes, shard, nw)
        nc_b = build_B(n_nodes, shard, nw, tiles_w, Tpad)
        nc_c = build_C(n_nodes, shard, nw, tiles_w, Tpad)
        _PROG_CACHE[ck] = (nc_a, nc_b, nc_c)

    # ---------------- launch A
    cident = np.eye(128, dtype=np.float32)
    ciota = np.tile(np.arange(128, dtype=np.float32), (128, 1))
    consts = {"cident": cident, "ciota": ciota}
    in_a = [{"xs": pad_shard(x, k, D), "vcat": vcat, **consts}
            for k in range(NC_N)]
    res_a = _run(nc_a, in_a)
    a1f = np.concatenate([res_a[k]["a1"][:shard] for k in range(NC_N)], 0)
    csums = np.stack([res_a[k]["csum"][0] for k in range(NC_N)], 0)
    DEBUG.update(a1f=a1f, csums=csums)

    # ---------------- launch B
    ones = np.ones((n_nodes, 1), np.float32)
    tx = np.concatenate([x, ones], axis=1).astype(np.float32)
    common_b = {
        "tx": tx, "csums": csums.astype(np.float32),
        "gw1": np.asarray(gate_w1, np.float32),
        "gb1": np.asarray(gate_b1, np.float32).reshape(1, D),
        "gw2": np.asarray(gate_w2, np.float32),
        "gb2": np.asarray(gate_b2, np.float32).reshape(1, 3),
        "gcn_w1": np.asarray(gcn_w1, np.float32),
        "gcn1_s": gcn1_s, "gcn1_b": gcn1_b,
        "sage_wl1": np.asarray(sage_wl1, np.float32),
        "sage_wr1": np.asarray(sage_wr1, np.float32),
        "sage_bl1": np.asarray(sage_bl1, np.float32).reshape(D, 1),
        "w2A": np.asarray(gat_w2, np.float32)[:128],
        "w2B": np.asarray(gat_w2, np.float32)[128:],
        "v2u2": v2u2,
        "w1h": np.asarray(gat_w1, np.float32),
        "b1c": np.asarray(gat_b1, np.float32).reshape(2, 128).T.copy(),
        **consts,
    }
    in_b = []
    for k in range(NC_N):
        m = dict(common_b)
        m.update(streams[k])
        m["xs_pad"] = pad_shard(x, k, D)
        # host-side halo routing: per-edge attention scalars (pure gather)
        m["sa"] = np.ascontiguousarray(
            a1f[streams[k]["idx_row"], :H1].reshape(128, Tpad * H1))
        m["sb"] = np.ascontiguousarray(
            a1f[streams[k]["idx_col"], H1:].reshape(128, Tpad * H1))
        in_b.append(m)
    res_b = _run(nc_b, in_b)
    h1f = np.concatenate([res_b[k]["h1"][:shard] for k in range(NC_N)], 0)
    hsf = np.concatenate([res_b[k]["hs1"][:shard] for k in range(NC_N)], 0)
    h2f = np.concatenate([res_b[k]["h2"][:shard] for k in range(NC_N)], 0)
    a2f = np.concatenate([res_b[k]["a2"][:shard] for k in range(NC_N)], 0)
    gate = res_b[0]["gate"].astype(np.float32)
    DEBUG.update(h1f=h1f, hsf=hsf, h2f=h2f, a2f=a2f, gate=gate)

    # ---------------- launch C
    tcat = np.concatenate(
        [h1f, h2f, np.ones((n_nodes, 1), np.float32), hsf],
        axis=1).astype(np.float32)
    common_c = {
        "tcat": tcat,
        "gcn_w2": np.asarray(gcn_w2, np.float32),
        "gcn_b2c": np.asarray(gcn_b2, np.float32).reshape(D, 1),
        "sage_wl2": np.asarray(sage_wl2, np.float32),
        "sage_wr2": np.asarray(sage_wr2, np.float32),
        "sage_bl2c": np.asarray(sage_bl2, np.float32).reshape(D, 1),
        "gat_b2r": np.asarray(gat_b2, np.float32).reshape(1, D),
        "gate": gate,
        **consts,
    }
    in_c = []
    for k in range(NC_N):
        m = dict(common_c)
        m.update(streams[k])
        m["hs_pad"] = pad_shard(hsf, k, D)
        m["sa2"] = np.ascontiguousarray(a2f[streams[k]["idx_row"], 0])
        m["sb2"] = np.ascontiguousarray(a2f[streams[k]["idx_col"], 1])
        in_c.append(m)
    res_c = _run(nc_c, in_c)
    out = np.concatenate([res_c[k]["out"][:shard] for k in range(NC_N)], 0)
    return out.astype(np.float32)


def kernel(**inputs):
    return gnn_forward(**inputs)



# revision 5
# speedup vs baseline: 5.4940x; 5.4940x over previous
"""AdaptiveGNN (GCN+GAT+SAGE mixture) on 8 Trainium2 NeuronCores — single
NEFF launch with on-device AllGather collectives.

Strategy (destination-sharded graph parallelism, one program):
 - Core k owns nodes [k*6250, (k+1)*6250); edges (plus self-loops) are
   sorted by destination on the host into a static per-window tile
   schedule shared by all 8 cores (window = 128 destination rows).
 - Phase A (local): GAT attention projections a1 = x @ [v_src|v_dst] and
   column sums of x. Writes [x | 1 | a_src] rows plus a trailing
   column-sum row into a per-core DRAM buffer.
 - AllGather #1 ([6273, 69] per core -> [50184, 69]): every core now has
   the full graph's source features + attention sources (halo exchange).
 - Gate MLP computed redundantly on every core from the 8 column-sum rows.
 - Phase B: layer 1 of all three branches for the local destination
   shard. Per edge-tile: indirect-DMA gather of source rows from the
   AllGathered buffer, local gather of a_dst rows, one-hot "selection"
   matmuls accumulate segment sums in PSUM. Window tails produce
   h1 = relu(bn(gcn1)), h2 = elu(gat1) @ W2 (+ attn scalars), hs = sage1;
   all written into the second per-core DRAM buffer [h1|h2|1|hs|a2src].
 - AllGather #2 ([6273, 194] per core -> [50184, 194]).
 - Phase C: layer 2 of all three branches + gated mix -> final rows.
 - Host sends only the x shard, edge streams and weights (~4MB/core)
   and receives the per-core output rows; one PJRT dispatch total.
"""

import sys

sys.path.insert(0, "/opt/trn_rl_repo")

import numpy as np

from concourse import bacc, bass, mybir, tile
from concourse.bass_utils import run_bass_kernel_spmd
import concourse.tile_sem_assignment as _tsa

# Clamp Tile's DMA-completion semaphore lanes (kernel-tail Drain waits on
# every producer semaphore; walrus rejects instructions with too many
# sync waits).
_tsa.NUM_HWDGE_SEMS = 8
_tsa.NUM_SWDGE_GLOBAL_SEMS = 8

F32 = mybir.dt.float32
I32 = mybir.dt.int32
AF = mybir.ActivationFunctionType
ALU = mybir.AluOpType

NC_N = 8          # cores
D = 64            # feature dim
H1 = 4            # GAT hidden heads
WB = D + 1 + H1   # phase-B gather row: [x | 1 | a_src]            = 69
WC = 3 * D + 2    # phase-C gather row: [h1 | h2 | 1 | hs | a2src] = 194
NEG_SLOPE = 0.2
BN_EPS = 1e-5


# ----------------------------------------------------------------- host prep
def build_schedule(edge_index, n_nodes):
    """Sort edges (plus self-loops) by destination, shard by destination,
    and produce a tile schedule common to all cores plus per-core streams.
    Source indices are remapped into the AllGather row space
    (node n -> (n // shard) * (npad + 1) + n % shard)."""
    shard = n_nodes // NC_N
    nw = (shard + 127) // 128
    # per-core AllGather rows: +1 csum row, +1 pad so the collective's
    # element count stays even (NRT needs 8-byte-aligned collective sizes)
    nprow = nw * 128 + 2
    row = edge_index[0].astype(np.int64)
    col = edge_index[1].astype(np.int64)
    loops = np.arange(n_nodes, dtype=np.int64)
    r_all = np.concatenate([row, loops])
    c_all = np.concatenate([col, loops])

    # GCN symmetric normalization (self-loops included)
    deg = np.bincount(c_all, minlength=n_nodes).astype(np.float64)
    dis = np.where(deg > 0, deg ** -0.5, 0.0)
    wnorm_all = (dis[r_all] * dis[c_all]).astype(np.float32)
    # SAGE mean weights (real edges only; zero on appended self-loops)
    cnt = np.bincount(col, minlength=n_nodes).astype(np.float64)
    ws = (1.0 / np.maximum(cnt, 1.0))[col].astype(np.float32)
    wsage_all = np.concatenate([ws, np.zeros(n_nodes, np.float32)])
    # source node id -> AllGathered row
    rg_all = ((r_all // shard) * nprow + (r_all % shard)).astype(np.int64)

    per_core = []
    counts = np.zeros((NC_N, nw), dtype=np.int64)
    for k in range(NC_N):
        lo, hi = k * shard, (k + 1) * shard
        sel = np.nonzero((c_all >= lo) & (c_all < hi))[0]
        cl = c_all[sel] - lo
        order = np.argsort(cl, kind="stable")
        sel = sel[order]
        cl = cl[order]
        w_of = cl // 128
        cnts = np.bincount(w_of, minlength=nw)
        counts[k] = cnts
        per_core.append((sel, cl, cnts))

    tiles_w = np.maximum(1, (counts.max(axis=0) + 127) // 128)
    T = int(tiles_w.sum())

    streams = []
    for k in range(NC_N):
        sel, cl, cnts = per_core[k]
        idx_row = np.zeros(T * 128, np.int32)
        idx_colL = np.zeros(T * 128, np.int32)
        colrel = np.full(T * 128, -1.0, np.float32)
        wnorm = np.zeros(T * 128, np.float32)
        wsage = np.zeros(T * 128, np.float32)
        pos = 0      # position in padded stream
        epos = 0     # position in this core's sorted edge list
        for w in range(nw):
            cw = int(cnts[w])
            seg = sel[epos:epos + cw]
            base = pos
            idx_row[base:base + cw] = rg_all[seg]
            idx_colL[base:base + cw] = cl[epos:epos + cw]
            colrel[base:base + cw] = (cl[epos:epos + cw] % 128).astype(np.float32)
            wnorm[base:base + cw] = wnorm_all[seg]
            wsage[base:base + cw] = wsage_all[seg]
            epos += cw
            pos += int(tiles_w[w]) * 128
        st = {
            "idx_row": idx_row.reshape(T, 128).T.copy(),
            "idx_colL": idx_colL.reshape(T, 128).T.copy(),
            "colrel": colrel.reshape(T, 128).T.copy(),
            "wnorm": wnorm.reshape(T, 128).T.copy(),
            "wsage": wsage.reshape(T, 128).T.copy(),
        }
        streams.append(st)
    return streams, [int(t) for t in tiles_w], T, shard, nw


# ------------------------------------------------------------- device pieces
def _load_w(nc, pool, dram, shape, tag, in_dtype=None):
    ld = pool.tile(list(shape), in_dtype or F32, tag=tag + "_ld")
    nc.sync.dma_start(out=ld[:], in_=dram[:])
    t = pool.tile(list(shape), F32, tag=tag)
    nc.vector.tensor_copy(t[:], ld[:])
    return t


# ----------------------------------------------------------------- the build
def build_full(n_nodes, tiles_w, T):
    shard = n_nodes // NC_N
    nw = (shard + 127) // 128
    npad = nw * 128
    nprow = npad + 2   # +1 csum row, +1 pad row (8-byte collective align)
    gfull = NC_N * nprow
    rem = shard - (nw - 1) * 128       # rows in last output window

    nc = bacc.Bacc()
    dr = {}
    for nm, shp, dt in [
        ("xs65", [npad, D + 1], F32),
        ("idx_row", [128, T], I32), ("idx_colL", [128, T], I32),
        ("colrel", [128, T], F32), ("wnorm", [128, T], F32),
        ("wsage", [128, T], F32),
        ("vcat", [D, 2 * H1], F32),
        ("gw1", [D, D], F32), ("gb1", [1, D], F32),
        ("gw2", [D, 3], F32), ("gb2", [1, 3], F32),
        ("gcn_w1", [D, D], F32), ("gcn1_s", [D, 1], F32), ("gcn1_b", [D, 1], F32),
        ("sage_wl1", [D, D], F32), ("sage_wr1", [D, D], F32),
        ("sage_bl1", [D, 1], F32),
        ("w2A", [128, D], F32), ("w2B", [128, D], F32),
        ("v2u2", [128, 4], F32), ("w1h", [D, 4 * D], F32),
        ("b1c", [128, 2], F32),
        ("gcn_w2", [D, D], F32), ("gcn_b2c", [D, 1], F32),
        ("sage_wl2", [D, D], F32), ("sage_wr2", [D, D], F32),
        ("sage_bl2c", [D, 1], F32), ("gat_b2r", [1, D], F32),
    ]:
        dr[nm] = nc.dram_tensor(nm, shp, dt, kind="ExternalInput")
    out = nc.dram_tensor("out", [shard, D], F32, kind="ExternalOutput")
    c_ident = nc.inline_tensor(np.eye(128, dtype=np.float32), name="cident")
    c_iota = nc.inline_tensor(
        np.tile(np.arange(128, dtype=np.float32), (128, 1)), name="ciota")

    with tile.TileContext(nc) as tc:
        with (
            tc.tile_pool(name="const", bufs=1) as const,
            tc.tile_pool(name="wts", bufs=1) as wts,
            tc.tile_pool(name="stream", bufs=1) as stream,
            tc.tile_pool(name="stage", bufs=1) as stage,
            tc.tile_pool(name="dramp", bufs=1, space="DRAM") as dramp,
        ):
            ident = _load_w(nc, const, c_ident, (128, 128), "ident")
            iota_f = _load_w(nc, const, c_iota, (128, 128), "iota_f")
            ones_col = const.tile([128, 1], F32, tag="ones_col")
            nc.vector.memset(ones_col[:], 1.0)
            ones_row = const.tile([1, 128], F32, tag="ones_row")
            nc.vector.memset(ones_row[:], 1.0)

            # DRAM buffers for the halo exchange
            bufB = dramp.tile([nprow, WB], F32, tag="bufB")
            gathB = dramp.tile([gfull, WB], F32, tag="gathB")
            bufC = dramp.tile([nprow, WC], F32, tag="bufC")
            gathC = dramp.tile([gfull, WC], F32, tag="gathC")
            adst = dramp.tile([npad, H1], F32, tag="adst")
            a2dst = dramp.tile([npad, 1], F32, tag="a2dst")

            # ---- weights to SBUF
            W = {}
            for nm, shp in [
                ("vcat", (D, 2 * H1)),
                ("gw1", (D, D)), ("gb1", (1, D)), ("gw2", (D, 3)), ("gb2", (1, 3)),
                ("gcn_w1", (D, D)), ("gcn1_s", (D, 1)), ("gcn1_b", (D, 1)),
                ("sage_wl1", (D, D)), ("sage_wr1", (D, D)), ("sage_bl1", (D, 1)),
                ("w2A", (128, D)), ("w2B", (128, D)), ("v2u2", (128, 4)),
                ("w1h", (D, 4 * D)), ("b1c", (128, 2)),
                ("gcn_w2", (D, D)), ("gcn_b2c", (D, 1)),
                ("sage_wl2", (D, D)), ("sage_wr2", (D, D)),
                ("sage_bl2c", (D, 1)), ("gat_b2r", (1, D)),
            ]:
                W[nm] = _load_w(nc, wts, dr[nm], shp, nm)

            # ---- streams to SBUF
            S = {}
            for nm, dt in [("idx_row", I32), ("idx_colL", I32), ("colrel", F32),
                           ("wnorm", F32), ("wsage", F32)]:
                t = stream.tile([128, T], dt, tag=nm)
                nc.sync.dma_start(out=t[:], in_=dr[nm][:])
                S[nm] = t

            # ---- staging buffers (SBUF-resident across phases)
            st_hs = stage.tile([128, nw * D], F32, tag="st_hs")
            st_out = stage.tile([128, nw * D], F32, tag="st_out")

            # ================================================== phase A
            with (
                tc.tile_pool(name="sbA", bufs=3) as sbA,
                tc.tile_pool(name="psA", bufs=2, space="PSUM") as psA,
                tc.tile_pool(name="pcsA", bufs=1, space="PSUM") as pcsA,
            ):
                csum_p = pcsA.tile([1, D], F32, tag="csum")
                for w in range(nw):
                    xt0 = sbA.tile([128, D + 1], F32, tag="xt0")
                    nc.sync.dma_start(out=xt0[:],
                                      in_=dr["xs65"][w * 128:(w + 1) * 128, :])
                    xt = sbA.tile([128, D + 1], F32, tag="xt")
                    nc.vector.tensor_copy(xt[:], xt0[:])
                    nc.sync.dma_start(
                        out=bufB[w * 128:(w + 1) * 128, 0:D + 1], in_=xt[:])
                    pT = psA.tile([D, 128], F32, tag="pT")
                    nc.tensor.matmul(out=pT[:], lhsT=xt[:, 0:D], rhs=ident[:],
                                     is_transpose=True)
                    xT = sbA.tile([D, 128], F32, tag="xT")
                    nc.vector.tensor_copy(xT[:], pT[:])
                    pa = psA.tile([2 * H1, 128], F32, tag="pa")
                    nc.tensor.matmul(out=pa[:], lhsT=W["vcat"][:], rhs=xT[:])
                    aT = sbA.tile([2 * H1, 128], F32, tag="aT")
                    nc.vector.tensor_copy(aT[:], pa[:])
                    pb = psA.tile([128, 2 * H1], F32, tag="pb")
                    nc.tensor.matmul(out=pb[:], lhsT=aT[:],
                                     rhs=ident[:2 * H1, :2 * H1],
                                     is_transpose=True)
                    ab = sbA.tile([128, 2 * H1], F32, tag="ab")
                    nc.vector.tensor_copy(ab[:], pb[:])
                    nc.sync.dma_start(
                        out=bufB[w * 128:(w + 1) * 128, D + 1:WB],
                        in_=ab[:, 0:H1])
                    nc.sync.dma_start(
                        out=adst[w * 128:(w + 1) * 128, :], in_=ab[:, H1:2 * H1])
                    nc.tensor.matmul(out=csum_p[:], lhsT=ones_col[:],
                                     rhs=xt[:, 0:D],
                                     start=(w == 0), stop=(w == nw - 1))
                cs = sbA.tile([1, D], F32, tag="cs")
                nc.vector.tensor_copy(cs[:], csum_p[:])
                nc.sync.dma_start(out=bufB[npad:npad + 1, 0:D], in_=cs[:])

            # ============================================== AllGather #1
            nc.gpsimd.collective_compute(
                "AllGather", ALU.bypass,
                replica_groups=[list(range(NC_N))],
                ins=[bufB[:].opt()], outs=[gathB[:].opt()])

            # ================================================== phase B
            with (
                tc.tile_pool(name="gatB", bufs=8) as gat,
                tc.tile_pool(name="mB", bufs=8) as mpool,
                tc.tile_pool(name="smB", bufs=3) as sm,
                tc.tile_pool(name="tlB", bufs=4) as tl,
                tc.tile_pool(name="paccB", bufs=1, space="PSUM") as pacc,
                tc.tile_pool(name="ptmpB", bufs=2, space="PSUM") as ptmp,
            ):
                # ---- gate MLP from the 8 AllGathered csum rows
                cs8l = sm.tile([NC_N, D], F32, tag="g_cs8l")
                for k in range(NC_N):
                    nc.sync.dma_start(
                        out=cs8l[k:k + 1, :],
                        in_=gathB[k * nprow + npad:k * nprow + npad + 1, 0:D])
                cs8 = sm.tile([NC_N, D], F32, tag="g_cs8")
                nc.vector.tensor_copy(cs8[:], cs8l[:])
                pxb = ptmp.tile([1, D], F32, tag="pt")
                nc.tensor.matmul(out=pxb[:], lhsT=ones_col[:NC_N, :1],
                                 rhs=cs8[:])
                xbar = sm.tile([1, D], F32, tag="g_xbar")
                nc.vector.tensor_scalar(out=xbar[:], in0=pxb[:],
                                        scalar1=1.0 / n_nodes, scalar2=None,
                                        op0=ALU.mult)
                pxT = ptmp.tile([D, 1], F32, tag="pt")
                nc.tensor.matmul(out=pxT[:], lhsT=xbar[:], rhs=ident[:1, :1],
                                 is_transpose=True)
                xbT = sm.tile([D, 1], F32, tag="g_xbT")
                nc.vector.tensor_copy(xbT[:], pxT[:])
                pg1 = ptmp.tile([1, D], F32, tag="pt")
                nc.tensor.matmul(out=pg1[:], lhsT=xbT[:], rhs=W["gw1"][:])
                g1 = sm.tile([1, D], F32, tag="g_g1")
                nc.vector.tensor_tensor(out=g1[:], in0=pg1[:], in1=W["gb1"][:],
                                        op=ALU.add)
                g1r = sm.tile([1, D], F32, tag="g_g1r")
                nc.vector.tensor_scalar(out=g1r[:], in0=g1[:], scalar1=0.0,
                                        scalar2=None, op0=ALU.max)
                pg1T = ptmp.tile([D, 1], F32, tag="pt")
                nc.tensor.matmul(out=pg1T[:], lhsT=g1r[:], rhs=ident[:1, :1],
                                 is_transpose=True)
                g1T = sm.tile([D, 1], F32, tag="g_g1T")
                nc.vector.tensor_copy(g1T[:], pg1T[:])
                pg2 = ptmp.tile([1, 3], F32, tag="pt")
                nc.tensor.matmul(out=pg2[:], lhsT=g1T[:], rhs=W["gw2"][:])
                g2 = sm.tile([1, 3], F32, tag="g_g2")
                nc.vector.tensor_tensor(out=g2[:], in0=pg2[:], in1=W["gb2"][:],
                                        op=ALU.add)
                g2e = sm.tile([1, 3], F32, tag="g_g2e")
                nc.scalar.activation(out=g2e[:], in_=g2[:], func=AF.Exp)
                g2s = sm.tile([1, 1], F32, tag="g_g2s")
                nc.vector.tensor_reduce(out=g2s[:], in_=g2e[:],
                                        axis=mybir.AxisListType.X, op=ALU.add)
                g2r = sm.tile([1, 1], F32, tag="g_g2r")
                nc.vector.reciprocal(g2r[:], g2s[:])
                gate_sb = sm.tile([1, 3], F32, tag="g_gate")
                nc.vector.tensor_scalar(out=gate_sb[:], in0=g2e[:],
                                        scalar1=g2r[:, :1], scalar2=None,
                                        op0=ALU.mult)
                # gate scalar broadcasts (used by phase C tails)
                pw128 = ptmp.tile([128, 3], F32, tag="pt")
                nc.tensor.matmul(out=pw128[:], lhsT=ones_row[:], rhs=gate_sb[:])
                wc = wts.tile([128, 3], F32, tag="wc")
                nc.vector.tensor_copy(wc[:], pw128[:])
                pw64 = ptmp.tile([D, 3], F32, tag="pt")
                nc.tensor.matmul(out=pw64[:], lhsT=ones_row[:1, :D],
                                 rhs=gate_sb[:])
                w64 = wts.tile([D, 3], F32, tag="w64")
                nc.vector.tensor_copy(w64[:], pw64[:])
                b2w0 = wts.tile([D, 1], F32, tag="b2w0")
                nc.vector.tensor_scalar(out=b2w0[:], in0=W["gcn_b2c"][:],
                                        scalar1=w64[:, 0:1], scalar2=None,
                                        op0=ALU.mult)
                pbg = ptmp.tile([128, D], F32, tag="pt")
                nc.tensor.matmul(out=pbg[:], lhsT=ones_row[:],
                                 rhs=W["gat_b2r"][:])
                bgat = wts.tile([128, D], F32, tag="bgat")
                nc.vector.tensor_scalar(out=bgat[:], in0=pbg[:],
                                        scalar1=wc[:, 1:2], scalar2=None,
                                        op0=ALU.mult)

                # ---- edge loop
                t_glob = 0
                for w in range(nw):
                    ntw = tiles_w[w]
                    p_gcnT = pacc.tile([D, 128], F32, tag="p_gcnT")
                    p_sageT = pacc.tile([D, 128], F32, tag="p_sageT")
                    p_gath = []
                    for h in range(H1):
                        pg = pacc.tile([128, D + 1], F32, tag=f"p_gat{h}")
                        p_gath.append(pg)
                    for t in range(ntw):
                        Gt = gat.tile([128, WB], F32, tag="G")
                        nc.gpsimd.indirect_dma_start(
                            out=Gt[:], out_offset=None, in_=gathB[:],
                            in_offset=bass.IndirectOffsetOnAxis(
                                ap=S["idx_row"][:, t_glob:t_glob + 1], axis=0))
                        Gc = gat.tile([128, WB], F32, tag="Gc")
                        nc.vector.tensor_copy(Gc[:], Gt[:])
                        sbt = gat.tile([128, H1], F32, tag="sbt")
                        nc.gpsimd.indirect_dma_start(
                            out=sbt[:], out_offset=None, in_=adst[:],
                            in_offset=bass.IndirectOffsetOnAxis(
                                ap=S["idx_colL"][:, t_glob:t_glob + 1], axis=0))
                        zt = gat.tile([128, H1], F32, tag="z")
                        nc.vector.tensor_tensor(
                            out=zt[:], in0=Gc[:, D + 1:WB], in1=sbt[:],
                            op=ALU.add)
                        zs = gat.tile([128, H1], F32, tag="zs")
                        nc.vector.tensor_scalar(out=zs[:], in0=zt[:],
                                                scalar1=NEG_SLOPE, scalar2=None,
                                                op0=ALU.mult)
                        nc.vector.tensor_tensor(out=zt[:], in0=zt[:], in1=zs[:],
                                                op=ALU.max)
                        et = gat.tile([128, H1], F32, tag="E")
                        nc.scalar.activation(out=et[:], in_=zt[:], func=AF.Exp)

                        g64 = Gc[:, 0:D]
                        g65 = Gc[:, 0:D + 1]
                        cr = S["colrel"][:, t_glob:t_glob + 1]
                        st, sp = (t == 0), (t == ntw - 1)
                        Mg = mpool.tile([128, 128], F32, tag="Mg")
                        nc.vector.tensor_scalar(
                            out=Mg[:], in0=iota_f[:], scalar1=cr,
                            scalar2=S["wnorm"][:, t_glob:t_glob + 1],
                            op0=ALU.is_equal, op1=ALU.mult)
                        nc.tensor.matmul(out=p_gcnT[:], lhsT=g64, rhs=Mg[:],
                                         start=st, stop=sp)
                        Ms = mpool.tile([128, 128], F32, tag="Ms")
                        nc.vector.tensor_scalar(
                            out=Ms[:], in0=iota_f[:], scalar1=cr,
                            scalar2=S["wsage"][:, t_glob:t_glob + 1],
                            op0=ALU.is_equal, op1=ALU.mult)
                        nc.tensor.matmul(out=p_sageT[:], lhsT=g64, rhs=Ms[:],
                                         start=st, stop=sp)
                        for h in range(H1):
                            Mh = mpool.tile([128, 128], F32, tag="Mh")
                            nc.vector.tensor_scalar(
                                out=Mh[:], in0=iota_f[:], scalar1=cr,
                                scalar2=et[:, h:h + 1],
                                op0=ALU.is_equal, op1=ALU.mult)
                            nc.tensor.matmul(
                                out=p_gath[h][:], lhsT=Mh[:], rhs=g65,
                                start=st, stop=sp)
                        t_glob += 1

                    # ---------- window tails ----------
                    rows = slice(w * 128, (w + 1) * 128)
                    # GCN1: h1 = relu(s*(W1^T aggT) + b)
                    aggT = tl.tile([D, 128], F32, tag="aggT")
                    nc.vector.tensor_copy(aggT[:], p_gcnT[:])
                    ph1T = ptmp.tile([D, 128], F32, tag="pt")
                    nc.tensor.matmul(out=ph1T[:], lhsT=W["gcn_w1"][:],
                                     rhs=aggT[:])
                    h1Ts = tl.tile([D, 128], F32, tag="h1Ts")
                    nc.scalar.activation(out=h1Ts[:], in_=ph1T[:], func=AF.Relu,
                                         scale=W["gcn1_s"][:, :1],
                                         bias=W["gcn1_b"][:, :1])
                    h1Tv = tl.tile([D, 128], F32, tag="h1Tv")
                    nc.vector.tensor_copy(h1Tv[:], h1Ts[:])
                    ph1 = ptmp.tile([128, D], F32, tag="pt")
                    nc.tensor.matmul(out=ph1[:], lhsT=h1Tv[:], rhs=ident[:D, :D],
                                     is_transpose=True)
                    h1sb = tl.tile([128, D], F32, tag="h1sb")
                    nc.vector.tensor_copy(h1sb[:], ph1[:])
                    nc.sync.dma_start(out=bufC[rows, 0:D], in_=h1sb[:])

                    # GAT1 heads -> x2T halves -> h2, a2
                    x2TA = tl.tile([128, 128], F32, tag="x2TA")
                    x2TB = tl.tile([128, 128], F32, tag="x2TB")
                    for h in range(H1):
                        rd = tl.tile([128, 1], F32, tag="rd")
                        nc.vector.reciprocal(rd[:], p_gath[h][:, D:D + 1])
                        hd_sb = tl.tile([128, D], F32, tag="hd_sb")
                        nc.vector.tensor_scalar(
                            out=hd_sb[:], in0=p_gath[h][:, 0:D],
                            scalar1=rd[:, :1], scalar2=None, op0=ALU.mult)
                        pht = ptmp.tile([D, 128], F32, tag="pt")
                        nc.tensor.matmul(out=pht[:], lhsT=hd_sb[:], rhs=ident[:],
                                         is_transpose=True)
                        hdT = tl.tile([D, 128], F32, tag="hdT_g")
                        nc.vector.tensor_copy(hdT[:], pht[:])
                        pxh = ptmp.tile([D, 128], F32, tag="pt")
                        nc.tensor.matmul(out=pxh[:],
                                         lhsT=W["w1h"][:, h * D:(h + 1) * D],
                                         rhs=hdT[:])
                        stgt = x2TA if h < 2 else x2TB
                        nc.vector.tensor_copy(
                            stgt[(h % 2) * D:(h % 2 + 1) * D, :], pxh[:])
                    x2T = []
                    for half, px in enumerate((x2TA, x2TB)):
                        yT = tl.tile([128, 128], F32, tag="yT")
                        nc.vector.tensor_scalar(
                            out=yT[:], in0=px[:],
                            scalar1=W["b1c"][:, half:half + 1], scalar2=None,
                            op0=ALU.add)
                        ymin = tl.tile([128, 128], F32, tag="ymin")
                        nc.vector.tensor_scalar(out=ymin[:], in0=yT[:],
                                                scalar1=0.0, scalar2=None,
                                                op0=ALU.min)
                        yexp = tl.tile([128, 128], F32, tag="yexp")
                        nc.scalar.activation(out=yexp[:], in_=ymin[:],
                                             func=AF.Exp)
                        ye1 = tl.tile([128, 128], F32, tag="ye1")
                        nc.vector.tensor_scalar(out=ye1[:], in0=yexp[:],
                                                scalar1=-1.0, scalar2=None,
                                                op0=ALU.add)
                        ymax = tl.tile([128, 128], F32, tag="ymax")
                        nc.vector.tensor_scalar(out=ymax[:], in0=yT[:],
                                                scalar1=0.0, scalar2=None,
                                                op0=ALU.max)
                        xt2 = tl.tile([128, 128], F32, tag=f"x2T{half}")
                        nc.vector.tensor_tensor(out=xt2[:], in0=ymax[:],
                                                in1=ye1[:], op=ALU.add)
                        x2T.append(xt2)
                    ph2T = ptmp.tile([D, 128], F32, tag="pt")
                    nc.tensor.matmul(out=ph2T[:], lhsT=W["w2A"][:],
                                     rhs=x2T[0][:], start=True, stop=False)
                    nc.tensor.matmul(out=ph2T[:], lhsT=W["w2B"][:],
                                     rhs=x2T[1][:], start=False, stop=True)
                    pa2T = ptmp.tile([2, 128], F32, tag="pt")
                    nc.tensor.matmul(out=pa2T[:], lhsT=W["v2u2"][:, 0:2],
                                     rhs=x2T[0][:], start=True, stop=False)
                    nc.tensor.matmul(out=pa2T[:], lhsT=W["v2u2"][:, 2:4],
                                     rhs=x2T[1][:], start=False, stop=True)
                    h2Ts = tl.tile([D, 128], F32, tag="h2Ts")
                    nc.vector.tensor_copy(h2Ts[:], ph2T[:])
                    ph2 = ptmp.tile([128, D], F32, tag="pt")
                    nc.tensor.matmul(out=ph2[:], lhsT=h2Ts[:], rhs=ident[:D, :D],
                                     is_transpose=True)
                    h2sb = tl.tile([128, D], F32, tag="h2sb")
                    nc.vector.tensor_copy(h2sb[:], ph2[:])
                    nc.sync.dma_start(out=bufC[rows, D:2 * D], in_=h2sb[:])
                    nc.sync.dma_start(out=bufC[rows, 2 * D:2 * D + 1],
                                      in_=ones_col[:])
                    a2Ts = tl.tile([2, 128], F32, tag="a2Ts")
                    nc.vector.tensor_copy(a2Ts[:], pa2T[:])
                    pa2 = ptmp.tile([128, 2], F32, tag="pt")
                    nc.tensor.matmul(out=pa2[:], lhsT=a2Ts[:], rhs=ident[:2, :2],
                                     is_transpose=True)
                    a2sb = tl.tile([128, 2], F32, tag="a2sb")
                    nc.vector.tensor_copy(a2sb[:], pa2[:])
                    nc.sync.dma_start(out=bufC[rows, WC - 1:WC],
                                      in_=a2sb[:, 0:1])
                    nc.sync.dma_start(out=a2dst[rows, :], in_=a2sb[:, 1:2])

                    # SAGE1
                    meanT = tl.tile([D, 128], F32, tag="meanT")
                    nc.vector.tensor_copy(meanT[:], p_sageT[:])
                    xd0 = tl.tile([128, D], F32, tag="xd0")
                    nc.sync.dma_start(out=xd0[:], in_=dr["xs65"][rows, 0:D])
                    xd = tl.tile([128, D], F32, tag="xd")
                    nc.vector.tensor_copy(xd[:], xd0[:])
                    pxdT = ptmp.tile([D, 128], F32, tag="pt")
                    nc.tensor.matmul(out=pxdT[:], lhsT=xd[:], rhs=ident[:],
                                     is_transpose=True)
                    xdT = tl.tile([D, 128], F32, tag="xdT")
                    nc.vector.tensor_copy(xdT[:], pxdT[:])
                    psT = ptmp.tile([D, 128], F32, tag="pt")
                    nc.tensor.matmul(out=psT[:], lhsT=W["sage_wl1"][:],
                                     rhs=meanT[:], start=True, stop=False)
                    nc.tensor.matmul(out=psT[:], lhsT=W["sage_wr1"][:],
                                     rhs=xdT[:], start=False, stop=True)
                    sTs = tl.tile([D, 128], F32, tag="sTs")
                    nc.scalar.activation(out=sTs[:], in_=psT[:],
                                         func=AF.Identity,
                                         bias=W["sage_bl1"][:, :1])
                    sTv = tl.tile([D, 128], F32, tag="sTv")
                    nc.vector.tensor_copy(sTv[:], sTs[:])
                    ps_ = ptmp.tile([128, D], F32, tag="pt")
                    nc.tensor.matmul(out=ps_[:], lhsT=sTv[:], rhs=ident[:D, :D],
                                     is_transpose=True)
                    s_sb = tl.tile([128, D], F32, tag="s_sb")
                    nc.vector.tensor_copy(s_sb[:], ps_[:])
                    sq = tl.tile([128, D], F32, tag="sq")
                    nc.vector.tensor_tensor(out=sq[:], in0=s_sb[:], in1=s_sb[:],
                                            op=ALU.mult)
                    ssum = tl.tile([128, 1], F32, tag="ssum")
                    nc.vector.tensor_reduce(out=ssum[:], in_=sq[:],
                                            axis=mybir.AxisListType.X,
                                            op=ALU.add)
                    nc.vector.tensor_scalar(out=ssum[:], in0=ssum[:],
                                            scalar1=1e-24, scalar2=None,
                                            op0=ALU.add)
                    rs = tl.tile([128, 1], F32, tag="rs")
                    nc.vector.reciprocal(rs[:], ssum[:])
                    rq = tl.tile([128, 1], F32, tag="rq")
                    nc.scalar.activation(out=rq[:], in_=rs[:], func=AF.Sqrt)
                    nc.vector.tensor_scalar(out=st_hs[:, w * D:(w + 1) * D],
                                            in0=s_sb[:], scalar1=rq[:, :1],
                                            scalar2=0.0, op0=ALU.mult,
                                            op1=ALU.max)
                    nc.sync.dma_start(out=bufC[rows, 2 * D + 1:3 * D + 1],
                                      in_=st_hs[:, w * D:(w + 1) * D])

            # ============================================== AllGather #2
            nc.gpsimd.collective_compute(
                "AllGather", ALU.bypass,
                replica_groups=[list(range(NC_N))],
                ins=[bufC[:].opt()], outs=[gathC[:].opt()])

            # ================================================== phase C
            with (
                tc.tile_pool(name="gatC", bufs=8) as gat,
                tc.tile_pool(name="mC", bufs=8) as mpool,
                tc.tile_pool(name="tlC", bufs=4) as tl,
                tc.tile_pool(name="paccC", bufs=1, space="PSUM") as pacc,
                tc.tile_pool(name="ptmpC", bufs=2, space="PSUM") as ptmp,
            ):
                t_glob = 0
                for w in range(nw):
                    ntw = tiles_w[w]
                    p_g2T = pacc.tile([D, 128], F32, tag="p_g2T")
                    p_s2T = pacc.tile([D, 128], F32, tag="p_s2T")
                    p_gat2 = pacc.tile([128, D + 1], F32, tag="p_gat2")
                    for t in range(ntw):
                        Gt = gat.tile([128, WC], F32, tag="G2")
                        nc.gpsimd.indirect_dma_start(
                            out=Gt[:], out_offset=None, in_=gathC[:],
                            in_offset=bass.IndirectOffsetOnAxis(
                                ap=S["idx_row"][:, t_glob:t_glob + 1], axis=0))
                        Gc = gat.tile([128, WC], F32, tag="Gc2")
                        nc.vector.tensor_copy(Gc[:], Gt[:])
                        sbt = gat.tile([128, 1], F32, tag="sb2")
                        nc.gpsimd.indirect_dma_start(
                            out=sbt[:], out_offset=None, in_=a2dst[:],
                            in_offset=bass.IndirectOffsetOnAxis(
                                ap=S["idx_colL"][:, t_glob:t_glob + 1], axis=0))
                        z2 = gat.tile([128, 1], F32, tag="z2")
                        nc.vector.tensor_tensor(
                            out=z2[:], in0=Gc[:, WC - 1:WC], in1=sbt[:],
                            op=ALU.add)
                        z2s = gat.tile([128, 1], F32, tag="z2s")
                        nc.vector.tensor_scalar(out=z2s[:], in0=z2[:],
                                                scalar1=NEG_SLOPE, scalar2=None,
                                                op0=ALU.mult)
                        nc.vector.tensor_tensor(out=z2[:], in0=z2[:], in1=z2s[:],
                                                op=ALU.max)
                        e2 = gat.tile([128, 1], F32, tag="E2")
                        nc.scalar.activation(out=e2[:], in_=z2[:], func=AF.Exp)

                        g1s = Gc[:, 0:D]
                        g2s = Gc[:, D:2 * D + 1]
                        g3s = Gc[:, 2 * D + 1:3 * D + 1]
                        cr = S["colrel"][:, t_glob:t_glob + 1]
                        st, sp = (t == 0), (t == ntw - 1)
                        Mg = mpool.tile([128, 128], F32, tag="Mg")
                        nc.vector.tensor_scalar(
                            out=Mg[:], in0=iota_f[:], scalar1=cr,
                            scalar2=S["wnorm"][:, t_glob:t_glob + 1],
                            op0=ALU.is_equal, op1=ALU.mult)
                        nc.tensor.matmul(out=p_g2T[:], lhsT=g1s, rhs=Mg[:],
                                         start=st, stop=sp)
                        Ms = mpool.tile([128, 128], F32, tag="Ms")
                        nc.vector.tensor_scalar(
                            out=Ms[:], in0=iota_f[:], scalar1=cr,
                            scalar2=S["wsage"][:, t_glob:t_glob + 1],
                            op0=ALU.is_equal, op1=ALU.mult)
                        nc.tensor.matmul(out=p_s2T[:], lhsT=g3s, rhs=Ms[:],
                                         start=st, stop=sp)
                        Mh = mpool.tile([128, 128], F32, tag="Mh")
                        nc.vector.tensor_scalar(
                            out=Mh[:], in0=iota_f[:], scalar1=cr,
                            scalar2=e2[:, 0:1],
                            op0=ALU.is_equal, op1=ALU.mult)
                        nc.tensor.matmul(out=p_gat2[:], lhsT=Mh[:], rhs=g2s,
                                         start=st, stop=sp)
                        t_glob += 1

                    # ---------- window tails ----------
                    # GCN2 (+w0, +w0*b2)
                    aggT = tl.tile([D, 128], F32, tag="aggT")
                    nc.vector.tensor_copy(aggT[:], p_g2T[:])
                    poT = ptmp.tile([D, 128], F32, tag="pt")
                    nc.tensor.matmul(out=poT[:], lhsT=W["gcn_w2"][:],
                                     rhs=aggT[:])
                    oTs = tl.tile([D, 128], F32, tag="oTs")
                    nc.scalar.activation(out=oTs[:], in_=poT[:],
                                         func=AF.Identity,
                                         scale=w64[:, 0:1], bias=b2w0[:, :1])
                    oTv = tl.tile([D, 128], F32, tag="oTv")
                    nc.vector.tensor_copy(oTv[:], oTs[:])
                    po = ptmp.tile([128, D], F32, tag="pt")
                    nc.tensor.matmul(out=po[:], lhsT=oTv[:], rhs=ident[:D, :D],
                                     is_transpose=True)
                    ogcn = tl.tile([128, D], F32, tag="ogcn")
                    nc.vector.tensor_copy(ogcn[:], po[:])

                    # GAT2 (+w1)
                    rd = tl.tile([128, 1], F32, tag="rd")
                    nc.vector.reciprocal(rd[:], p_gat2[:, D:D + 1])
                    ogat = tl.tile([128, D], F32, tag="ogat")
                    nc.vector.tensor_scalar(out=ogat[:], in0=p_gat2[:, 0:D],
                                            scalar1=rd[:, :1],
                                            scalar2=wc[:, 1:2],
                                            op0=ALU.mult, op1=ALU.mult)

                    # SAGE2 (+w2); root rows come from the SBUF staging
                    meanT = tl.tile([D, 128], F32, tag="meanT")
                    nc.vector.tensor_copy(meanT[:], p_s2T[:])
                    phdT = ptmp.tile([D, 128], F32, tag="pt")
                    nc.tensor.matmul(out=phdT[:],
                                     lhsT=st_hs[:, w * D:(w + 1) * D],
                                     rhs=ident[:], is_transpose=True)
                    hdT = tl.tile([D, 128], F32, tag="hdT")
                    nc.vector.tensor_copy(hdT[:], phdT[:])
                    psT = ptmp.tile([D, 128], F32, tag="pt")
                    nc.tensor.matmul(out=psT[:], lhsT=W["sage_wl2"][:],
                                     rhs=meanT[:], start=True, stop=False)
                    nc.tensor.matmul(out=psT[:], lhsT=W["sage_wr2"][:],
                                     rhs=hdT[:], start=False, stop=True)
                    sTs = tl.tile([D, 128], F32, tag="sTs")
                    nc.scalar.activation(out=sTs[:], in_=psT[:],
                                         func=AF.Identity,
                                         bias=W["sage_bl2c"][:, :1])
                    sTv = tl.tile([D, 128], F32, tag="sTv")
                    nc.vector.tensor_copy(sTv[:], sTs[:])
                    ps_ = ptmp.tile([128, D], F32, tag="pt")
                    nc.tensor.matmul(out=ps_[:], lhsT=sTv[:], rhs=ident[:D, :D],
                                     is_transpose=True)
                    s_sb = tl.tile([128, D], F32, tag="s_sb")
                    nc.vector.tensor_copy(s_sb[:], ps_[:])
                    sq = tl.tile([128, D], F32, tag="sq")
                    nc.vector.tensor_tensor(out=sq[:], in0=s_sb[:], in1=s_sb[:],
                                            op=ALU.mult)
                    ssum = tl.tile([128, 1], F32, tag="ssum")
                    nc.vector.tensor_reduce(out=ssum[:], in_=sq[:],
                                            axis=mybir.AxisListType.X,
                                            op=ALU.add)
                    nc.vector.tensor_scalar(out=ssum[:], in0=ssum[:],
                                            scalar1=1e-24, scalar2=None,
                                            op0=ALU.add)
                    rs = tl.tile([128, 1], F32, tag="rs")
                    nc.vector.reciprocal(rs[:], ssum[:])
                    rq = tl.tile([128, 1], F32, tag="rq")
                    nc.scalar.activation(out=rq[:], in_=rs[:], func=AF.Sqrt)
                    osage = tl.tile([128, D], F32, tag="osage")
                    nc.vector.tensor_scalar(out=osage[:], in0=s_sb[:],
                                            scalar1=rq[:, :1],
                                            scalar2=wc[:, 2:3],
                                            op0=ALU.mult, op1=ALU.mult)

                    # mix
                    mx1 = tl.tile([128, D], F32, tag="mx1")
                    nc.vector.tensor_tensor(out=mx1[:], in0=ogcn[:],
                                            in1=ogat[:], op=ALU.add)
                    mx2 = tl.tile([128, D], F32, tag="mx2")
                    nc.vector.tensor_tensor(out=mx2[:], in0=mx1[:],
                                            in1=osage[:], op=ALU.add)
                    nc.vector.tensor_tensor(out=st_out[:, w * D:(w + 1) * D],
                                            in0=mx2[:], in1=bgat[:],
                                            op=ALU.add)

            # ---- final output DMA: full windows, then the partial tail
            out_ap = bass.AP(out, 0, [[D, 128], [128 * D, nw - 1], [1, D]])
            nc.sync.dma_start(
                out=out_ap,
                in_=st_out[:, 0:(nw - 1) * D].rearrange(
                    "p (w c) -> p w c", w=nw - 1))
            nc.sync.dma_start(
                out=out[(nw - 1) * 128:shard, :],
                in_=st_out[0:rem, (nw - 1) * D:nw * D])
    return nc


# ---------------------------------------------------------------- host logic
DEBUG = {}
_PROG_CACHE = {}


def _run(nc, in_maps, trace=False):
    import time as _time
    if not nc.is_finalized():
        nc.finalize()
    t0 = _time.perf_counter()
    res = run_bass_kernel_spmd(nc, in_maps, list(range(NC_N)), trace=trace)
    DEBUG.setdefault("run_walls", []).append(_time.perf_counter() - t0)
    if res.exec_time_ns:
        DEBUG.setdefault("exec_ns", []).append(res.exec_time_ns)
    return res.results


def gnn_forward(x, edge_index, gate_w1, gate_b1, gate_w2, gate_b2,
                gcn_w1, gcn_b1, bn_gamma, bn_beta, gcn_w2, gcn_b2,
                gat_w1, gat_att_src1, gat_att_dst1, gat_b1,
                gat_w2, gat_att_src2, gat_att_dst2, gat_b2,
                sage_wl1, sage_bl1, sage_wr1, sage_wl2, sage_bl2, sage_wr2,
                trace=False):
    n_nodes = x.shape[0]
    x = np.asarray(x, np.float32)
    streams, tiles_w, T, shard, nw = build_schedule(
        np.asarray(edge_index), n_nodes)
    npad = nw * 128

    # ---- host weight folding (weights only, no data)
    w1r = np.asarray(gat_w1, np.float32).reshape(D, H1, D)
    vsrc = np.einsum("chj,hj->ch", w1r, np.asarray(gat_att_src1, np.float32))
    vdst = np.einsum("chj,hj->ch", w1r, np.asarray(gat_att_dst1, np.float32))
    vcat = np.concatenate([vsrc, vdst], axis=1).astype(np.float32)  # [64,8]
    v2 = (np.asarray(gat_w2, np.float32) @
          np.asarray(gat_att_src2, np.float32)[0])  # [256]
    u2 = (np.asarray(gat_w2, np.float32) @
          np.asarray(gat_att_dst2, np.float32)[0])
    v2u2 = np.stack([v2[:128], u2[:128], v2[128:], u2[128:]],
                    axis=1).astype(np.float32)  # [128,4]
    bn_s = (np.asarray(bn_gamma, np.float32) /
            np.sqrt(np.float32(1.0 + BN_EPS)))
    gcn1_s = bn_s.reshape(D, 1).astype(np.float32)
    gcn1_b = (bn_s * np.asarray(gcn_b1, np.float32) +
              np.asarray(bn_beta, np.float32)).reshape(D, 1).astype(np.float32)

    ck = (n_nodes, T, tuple(tiles_w))
    if ck in _PROG_CACHE:
        nc = _PROG_CACHE[ck]
    else:
        nc = build_full(n_nodes, tiles_w, T)
        _PROG_CACHE[ck] = nc

    common = {
        "vcat": vcat,
        "gw1": np.asarray(gate_w1, np.float32),
        "gb1": np.asarray(gate_b1, np.float32).reshape(1, D),
        "gw2": np.asarray(gate_w2, np.float32),
        "gb2": np.asarray(gate_b2, np.float32).reshape(1, 3),
        "gcn_w1": np.asarray(gcn_w1, np.float32),
        "gcn1_s": gcn1_s, "gcn1_b": gcn1_b,
        "sage_wl1": np.asarray(sage_wl1, np.float32),
        "sage_wr1": np.asarray(sage_wr1, np.float32),
        "sage_bl1": np.asarray(sage_bl1, np.float32).reshape(D, 1),
        "w2A": np.asarray(gat_w2, np.float32)[:128],
        "w2B": np.asarray(gat_w2, np.float32)[128:],
        "v2u2": v2u2,
        "w1h": np.asarray(gat_w1, np.float32),
        "b1c": np.asarray(gat_b1, np.float32).reshape(2, 128).T.copy(),
        "gcn_w2": np.asarray(gcn_w2, np.float32),
        "gcn_b2c": np.asarray(gcn_b2, np.float32).reshape(D, 1),
        "sage_wl2": np.asarray(sage_wl2, np.float32),
        "sage_wr2": np.asarray(sage_wr2, np.float32),
        "sage_bl2c": np.asarray(sage_bl2, np.float32).reshape(D, 1),
        "gat_b2r": np.asarray(gat_b2, np.float32).reshape(1, D),
    }
    in_maps = []
    for k in range(NC_N):
        m = dict(common)
        m.update(streams[k])
        xs65 = np.zeros((npad, D + 1), np.float32)
        xs65[:shard, :D] = x[k * shard:(k + 1) * shard]
        xs65[:shard, D] = 1.0
        m["xs65"] = xs65
        in_maps.append(m)
    res = _run(nc, in_maps, trace=trace)
    out = np.concatenate([res[k]["out"] for k in range(NC_N)], 0)
    return out.astype(np.float32)


def kernel(**inputs):
    return gnn_forward(**inputs)


# revision 14
# speedup vs baseline: 6.0965x; 1.1097x over previous
"""AdaptiveGNN (GCN+GAT+SAGE mixture) on 8 Trainium2 NeuronCores — single
NEFF launch with on-device AllGather collectives.

Strategy (destination-sharded graph parallelism, one program):
 - Core k owns nodes [k*6250, (k+1)*6250); edges (plus self-loops) are
   sorted by destination on the host into a static per-window tile
   schedule shared by all 8 cores (window = 128 destination rows).
 - Phase A (local): GAT attention projections a1 = x @ [v_src|v_dst] and
   column sums of x. Writes [x | 1 | a_src] rows plus a trailing
   column-sum row into a per-core DRAM buffer.
 - AllGather #1 ([6273, 69] per core -> [50184, 69]): every core now has
   the full graph's source features + attention sources (halo exchange).
 - Gate MLP computed redundantly on every core from the 8 column-sum rows.
 - Phase B: layer 1 of all three branches for the local destination
   shard. Per edge-tile: indirect-DMA gather of source rows from the
   AllGathered buffer, local gather of a_dst rows, one-hot "selection"
   matmuls accumulate segment sums in PSUM. Window tails produce
   h1 = relu(bn(gcn1)), h2 = elu(gat1) @ W2 (+ attn scalars), hs = sage1;
   all written into the second per-core DRAM buffer [h1|h2|1|hs|a2src].
 - AllGather #2 ([6273, 194] per core -> [50184, 194]).
 - Phase C: layer 2 of all three branches + gated mix -> final rows.
 - Host sends only the x shard, edge streams and weights (~4MB/core)
   and receives the per-core output rows; one PJRT dispatch total.
"""

import sys

sys.path.insert(0, "/opt/trn_rl_repo")

import numpy as np

from concourse import bacc, bass, mybir, tile
from concourse.bass_utils import run_bass_kernel_spmd
import concourse.tile_sem_assignment as _tsa

# Clamp Tile's DMA-completion semaphore lanes (kernel-tail Drain waits on
# every producer semaphore; walrus rejects instructions with too many
# sync waits).
_tsa.NUM_HWDGE_SEMS = 8
_tsa.NUM_SWDGE_GLOBAL_SEMS = 8

F32 = mybir.dt.float32
F16 = mybir.dt.float16
I32 = mybir.dt.int32
AF = mybir.ActivationFunctionType
ALU = mybir.AluOpType

NC_N = 8          # cores
D = 64            # feature dim
H1 = 4            # GAT hidden heads
WB = D + 1 + H1   # phase-B gather row: [x | 1 | a_src]            = 69
WC = 3 * D + 2    # phase-C gather row: [h1 | h2 | 1 | hs | a2src] = 194
NEG_SLOPE = 0.2
BN_EPS = 1e-5


# ----------------------------------------------------------------- host prep
def build_schedule(edge_index, n_nodes):
    """Sort edges (plus self-loops) by destination, shard by destination,
    and produce a tile schedule common to all cores plus per-core streams.
    Source indices are remapped into the AllGather row space
    (node n -> (n // shard) * (npad + 1) + n % shard)."""
    shard = n_nodes // NC_N
    nw = (shard + 127) // 128
    # per-core AllGather rows: +1 csum row, +1 pad so the collective's
    # element count stays even (NRT needs 8-byte-aligned collective sizes)
    nprow = nw * 128 + 2
    row = edge_index[0].astype(np.int64)
    col = edge_index[1].astype(np.int64)
    loops = np.arange(n_nodes, dtype=np.int64)
    r_all = np.concatenate([row, loops])
    c_all = np.concatenate([col, loops])

    # GCN symmetric normalization (self-loops included)
    deg = np.bincount(c_all, minlength=n_nodes).astype(np.float64)
    dis = np.where(deg > 0, deg ** -0.5, 0.0)
    wnorm_all = (dis[r_all] * dis[c_all]).astype(np.float32)
    # SAGE mean weights (real edges only; zero on appended self-loops)
    cnt = np.bincount(col, minlength=n_nodes).astype(np.float64)
    ws = (1.0 / np.maximum(cnt, 1.0))[col].astype(np.float32)
    wsage_all = np.concatenate([ws, np.zeros(n_nodes, np.float32)])
    # source node id -> AllGathered row
    rg_all = ((r_all // shard) * nprow + (r_all % shard)).astype(np.int64)

    per_core = []
    counts = np.zeros((NC_N, nw), dtype=np.int64)
    for k in range(NC_N):
        lo, hi = k * shard, (k + 1) * shard
        sel = np.nonzero((c_all >= lo) & (c_all < hi))[0]
        cl = c_all[sel] - lo
        order = np.argsort(cl, kind="stable")
        sel = sel[order]
        cl = cl[order]
        w_of = cl // 128
        cnts = np.bincount(w_of, minlength=nw)
        counts[k] = cnts
        per_core.append((sel, cl, cnts))

    tiles_w = np.maximum(1, (counts.max(axis=0) + 127) // 128)
    T = int(tiles_w.sum())

    streams = []
    for k in range(NC_N):
        sel, cl, cnts = per_core[k]
        idx_row = np.zeros(T * 128, np.int32)
        idx_colL = np.zeros(T * 128, np.int32)
        colrel = np.full(T * 128, -1.0, np.float32)
        wnorm = np.zeros(T * 128, np.float32)
        wsage = np.zeros(T * 128, np.float32)
        pos = 0      # position in padded stream
        epos = 0     # position in this core's sorted edge list
        for w in range(nw):
            cw = int(cnts[w])
            seg = sel[epos:epos + cw]
            base = pos
            idx_row[base:base + cw] = rg_all[seg]
            idx_colL[base:base + cw] = cl[epos:epos + cw]
            colrel[base:base + cw] = (cl[epos:epos + cw] % 128).astype(np.float32)
            wnorm[base:base + cw] = wnorm_all[seg]
            wsage[base:base + cw] = wsage_all[seg]
            epos += cw
            pos += int(tiles_w[w]) * 128
        st = {
            "idx_row": idx_row.reshape(T, 128).T.copy(),
            "idx_colL": idx_colL.reshape(T, 128).T.copy(),
            "colrel": colrel.reshape(T, 128).T.astype(np.float16),
            "wnorm": wnorm.reshape(T, 128).T.astype(np.float16),
            "wsage": wsage.reshape(T, 128).T.astype(np.float16),
        }
        streams.append(st)
    return streams, [int(t) for t in tiles_w], T, shard, nw


# ------------------------------------------------------------- device pieces
def _load_w(nc, pool, dram, shape, tag, in_dtype=None):
    ld = pool.tile(list(shape), in_dtype or F32, tag=tag + "_ld")
    nc.sync.dma_start(out=ld[:], in_=dram[:])
    t = pool.tile(list(shape), F32, tag=tag)
    nc.vector.tensor_copy(t[:], ld[:])
    return t


# ----------------------------------------------------------------- the build
def build_full(n_nodes, tiles_w, T):
    shard = n_nodes // NC_N
    nw = (shard + 127) // 128
    npad = nw * 128
    nprow = npad + 2   # +1 csum row, +1 pad row (8-byte collective align)
    gfull = NC_N * nprow
    rem = shard - (nw - 1) * 128       # rows in last output window

    nc = bacc.Bacc()
    dr = {}
    for nm, shp, dt in [
        ("xs65", [npad, D + 1], F16),
        ("idx_row", [128, T], I32), ("idx_colL", [128, T], I32),
        ("colrel", [128, T], F16), ("wnorm", [128, T], F16),
        ("wsage", [128, T], F16),
        ("vcat", [D, 2 * H1], F32),
        ("gw1", [D, D], F32), ("gb1", [1, D], F32),
        ("gw2", [D, 3], F32), ("gb2", [1, 3], F32),
        ("gcn_w1", [D, D], F32), ("gcn1_s", [D, 1], F32), ("gcn1_b", [D, 1], F32),
        ("sage_wl1", [D, D], F32), ("sage_wr1", [D, D], F32),
        ("sage_bl1", [D, 1], F32),
        ("w2A", [128, D], F32), ("w2B", [128, D], F32),
        ("v2u2", [128, 4], F32), ("w1h", [D, 4 * D], F32),
        ("b1c", [128, 2], F32),
        ("gcn_w2", [D, D], F32), ("gcn_b2c", [D, 1], F32),
        ("sage_wl2", [D, D], F32), ("sage_wr2", [D, D], F32),
        ("sage_bl2c", [D, 1], F32), ("gat_b2r", [1, D], F32),
    ]:
        dr[nm] = nc.dram_tensor(nm, shp, dt, kind="ExternalInput")
    out = nc.dram_tensor("out", [shard, D], F16, kind="ExternalOutput")
    c_ident = nc.inline_tensor(np.eye(128, dtype=np.float32), name="cident")
    c_iota = nc.inline_tensor(
        np.tile(np.arange(128, dtype=np.float32), (128, 1)), name="ciota")

    with tile.TileContext(nc) as tc:
        with (
            tc.tile_pool(name="const", bufs=1) as const,
            tc.tile_pool(name="wts", bufs=1) as wts,
            tc.tile_pool(name="stream", bufs=1) as stream,
            tc.tile_pool(name="stage", bufs=1) as stage,
            tc.tile_pool(name="dramp", bufs=1, space="DRAM") as dramp,
        ):
            ident = _load_w(nc, const, c_ident, (128, 128), "ident")
            iota_f = _load_w(nc, const, c_iota, (128, 128), "iota_f")
            ones_col = const.tile([128, 1], F32, tag="ones_col")
            nc.vector.memset(ones_col[:], 1.0)
            ones_row = const.tile([1, 128], F32, tag="ones_row")
            nc.vector.memset(ones_row[:], 1.0)

            # DRAM buffers for the halo exchange
            bufB = dramp.tile([nprow, WB], F32, tag="bufB")
            gathB = dramp.tile([gfull, WB], F32, tag="gathB")
            bufC = dramp.tile([nprow, WC], F32, tag="bufC")
            gathC = dramp.tile([gfull, WC], F32, tag="gathC")
            adst = dramp.tile([npad, H1], F32, tag="adst")
            a2dst = dramp.tile([npad, 1], F32, tag="a2dst")

            # ---- weights to SBUF
            W = {}
            for nm, shp in [
                ("vcat", (D, 2 * H1)),
                ("gw1", (D, D)), ("gb1", (1, D)), ("gw2", (D, 3)), ("gb2", (1, 3)),
                ("gcn_w1", (D, D)), ("gcn1_s", (D, 1)), ("gcn1_b", (D, 1)),
                ("sage_wl1", (D, D)), ("sage_wr1", (D, D)), ("sage_bl1", (D, 1)),
                ("w2A", (128, D)), ("w2B", (128, D)), ("v2u2", (128, 4)),
                ("w1h", (D, 4 * D)), ("b1c", (128, 2)),
                ("gcn_w2", (D, D)), ("gcn_b2c", (D, 1)),
                ("sage_wl2", (D, D)), ("sage_wr2", (D, D)),
                ("sage_bl2c", (D, 1)), ("gat_b2r", (1, D)),
            ]:
                W[nm] = _load_w(nc, wts, dr[nm], shp, nm)

            # ---- streams to SBUF (f16 halves the tunnel bytes; convert once)
            S = {}
            for nm in ("idx_row", "idx_colL"):
                t = stream.tile([128, T], I32, tag=nm)
                nc.sync.dma_start(out=t[:], in_=dr[nm][:])
                S[nm] = t
            for nm in ("colrel", "wnorm", "wsage"):
                raw = stream.tile([128, T], F16, tag=nm + "_raw")
                nc.sync.dma_start(out=raw[:], in_=dr[nm][:])
                t = stream.tile([128, T], F32, tag=nm)
                nc.vector.tensor_copy(t[:], raw[:])
                S[nm] = t

            # ---- staging buffers (SBUF-resident across phases)
            st_hs = stage.tile([128, nw * D], F32, tag="st_hs")
            st_out = stage.tile([128, nw * D], F16, tag="st_out")

            # ================================================== phase A
            with (
                tc.tile_pool(name="sbA", bufs=3) as sbA,
                tc.tile_pool(name="psA", bufs=2, space="PSUM") as psA,
                tc.tile_pool(name="pcsA", bufs=1, space="PSUM") as pcsA,
            ):
                csum_p = pcsA.tile([1, D], F32, tag="csum")
                for w in range(nw):
                    xt0 = sbA.tile([128, D + 1], F16, tag="xt0")
                    nc.sync.dma_start(out=xt0[:],
                                      in_=dr["xs65"][w * 128:(w + 1) * 128, :])
                    xt = sbA.tile([128, D + 1], F32, tag="xt")
                    nc.vector.tensor_copy(xt[:], xt0[:])
                    nc.sync.dma_start(
                        out=bufB[w * 128:(w + 1) * 128, 0:D + 1], in_=xt[:])
                    pT = psA.tile([D, 128], F32, tag="pT")
                    nc.tensor.matmul(out=pT[:], lhsT=xt[:, 0:D], rhs=ident[:],
                                     is_transpose=True)
                    xT = sbA.tile([D, 128], F32, tag="xT")
                    nc.vector.tensor_copy(xT[:], pT[:])
                    pa = psA.tile([2 * H1, 128], F32, tag="pa")
                    nc.tensor.matmul(out=pa[:], lhsT=W["vcat"][:], rhs=xT[:])
                    aT = sbA.tile([2 * H1, 128], F32, tag="aT")
                    nc.vector.tensor_copy(aT[:], pa[:])
                    pb = psA.tile([128, 2 * H1], F32, tag="pb")
                    nc.tensor.matmul(out=pb[:], lhsT=aT[:],
                                     rhs=ident[:2 * H1, :2 * H1],
                                     is_transpose=True)
                    ab = sbA.tile([128, 2 * H1], F32, tag="ab")
                    nc.vector.tensor_copy(ab[:], pb[:])
                    nc.sync.dma_start(
                        out=bufB[w * 128:(w + 1) * 128, D + 1:WB],
                        in_=ab[:, 0:H1])
                    nc.sync.dma_start(
                        out=adst[w * 128:(w + 1) * 128, :], in_=ab[:, H1:2 * H1])
                    nc.tensor.matmul(out=csum_p[:], lhsT=ones_col[:],
                                     rhs=xt[:, 0:D],
                                     start=(w == 0), stop=(w == nw - 1))
                cs = sbA.tile([1, D], F32, tag="cs")
                nc.vector.tensor_copy(cs[:], csum_p[:])
                nc.sync.dma_start(out=bufB[npad:npad + 1, 0:D], in_=cs[:])

            # ============================================== AllGather #1
            nc.gpsimd.collective_compute(
                "AllGather", ALU.bypass,
                replica_groups=[list(range(NC_N))],
                ins=[bufB[:].opt()], outs=[gathB[:].opt()])

            # ================================================== phase B
            with (
                tc.tile_pool(name="gatB", bufs=8) as gat,
                tc.tile_pool(name="mB", bufs=8) as mpool,
                tc.tile_pool(name="smB", bufs=3) as sm,
                tc.tile_pool(name="tlB", bufs=4) as tl,
                tc.tile_pool(name="paccB", bufs=1, space="PSUM") as pacc,
                tc.tile_pool(name="ptmpB", bufs=2, space="PSUM") as ptmp,
            ):
                # ---- gate MLP from the 8 AllGathered csum rows
                cs8l = sm.tile([NC_N, D], F32, tag="g_cs8l")
                for k in range(NC_N):
                    nc.sync.dma_start(
                        out=cs8l[k:k + 1, :],
                        in_=gathB[k * nprow + npad:k * nprow + npad + 1, 0:D])
                cs8 = sm.tile([NC_N, D], F32, tag="g_cs8")
                nc.vector.tensor_copy(cs8[:], cs8l[:])
                pxb = ptmp.tile([1, D], F32, tag="pt")
                nc.tensor.matmul(out=pxb[:], lhsT=ones_col[:NC_N, :1],
                                 rhs=cs8[:])
                xbar = sm.tile([1, D], F32, tag="g_xbar")
                nc.vector.tensor_scalar(out=xbar[:], in0=pxb[:],
                                        scalar1=1.0 / n_nodes, scalar2=None,
                                        op0=ALU.mult)
                pxT = ptmp.tile([D, 1], F32, tag="pt")
                nc.tensor.matmul(out=pxT[:], lhsT=xbar[:], rhs=ident[:1, :1],
                                 is_transpose=True)
                xbT = sm.tile([D, 1], F32, tag="g_xbT")
                nc.vector.tensor_copy(xbT[:], pxT[:])
                pg1 = ptmp.tile([1, D], F32, tag="pt")
                nc.tensor.matmul(out=pg1[:], lhsT=xbT[:], rhs=W["gw1"][:])
                g1 = sm.tile([1, D], F32, tag="g_g1")
                nc.vector.tensor_tensor(out=g1[:], in0=pg1[:], in1=W["gb1"][:],
                                        op=ALU.add)
                g1r = sm.tile([1, D], F32, tag="g_g1r")
                nc.vector.tensor_scalar(out=g1r[:], in0=g1[:], scalar1=0.0,
                                        scalar2=None, op0=ALU.max)
                pg1T = ptmp.tile([D, 1], F32, tag="pt")
                nc.tensor.matmul(out=pg1T[:], lhsT=g1r[:], rhs=ident[:1, :1],
                                 is_transpose=True)
                g1T = sm.tile([D, 1], F32, tag="g_g1T")
                nc.vector.tensor_copy(g1T[:], pg1T[:])
                pg2 = ptmp.tile([1, 3], F32, tag="pt")
                nc.tensor.matmul(out=pg2[:], lhsT=g1T[:], rhs=W["gw2"][:])
                g2 = sm.tile([1, 3], F32, tag="g_g2")
                nc.vector.tensor_tensor(out=g2[:], in0=pg2[:], in1=W["gb2"][:],
                                        op=ALU.add)
                g2e = sm.tile([1, 3], F32, tag="g_g2e")
                nc.scalar.activation(out=g2e[:], in_=g2[:], func=AF.Exp)
                g2s = sm.tile([1, 1], F32, tag="g_g2s")
                nc.vector.tensor_reduce(out=g2s[:], in_=g2e[:],
                                        axis=mybir.AxisListType.X, op=ALU.add)
                g2r = sm.tile([1, 1], F32, tag="g_g2r")
                nc.vector.reciprocal(g2r[:], g2s[:])
                gate_sb = sm.tile([1, 3], F32, tag="g_gate")
                nc.vector.tensor_scalar(out=gate_sb[:], in0=g2e[:],
                                        scalar1=g2r[:, :1], scalar2=None,
                                        op0=ALU.mult)
                # gate scalar broadcasts (used by phase C tails)
                pw128 = ptmp.tile([128, 3], F32, tag="pt")
                nc.tensor.matmul(out=pw128[:], lhsT=ones_row[:], rhs=gate_sb[:])
                wc = wts.tile([128, 3], F32, tag="wc")
                nc.vector.tensor_copy(wc[:], pw128[:])
                pw64 = ptmp.tile([D, 3], F32, tag="pt")
                nc.tensor.matmul(out=pw64[:], lhsT=ones_row[:1, :D],
                                 rhs=gate_sb[:])
                w64 = wts.tile([D, 3], F32, tag="w64")
                nc.vector.tensor_copy(w64[:], pw64[:])
                b2w0 = wts.tile([D, 1], F32, tag="b2w0")
                nc.vector.tensor_scalar(out=b2w0[:], in0=W["gcn_b2c"][:],
                                        scalar1=w64[:, 0:1], scalar2=None,
                                        op0=ALU.mult)
                pbg = ptmp.tile([128, D], F32, tag="pt")
                nc.tensor.matmul(out=pbg[:], lhsT=ones_row[:],
                                 rhs=W["gat_b2r"][:])
                bgat = wts.tile([128, D], F32, tag="bgat")
                nc.vector.tensor_scalar(out=bgat[:], in0=pbg[:],
                                        scalar1=wc[:, 1:2], scalar2=None,
                                        op0=ALU.mult)

                # ---- edge loop
                t_glob = 0
                for w in range(nw):
                    ntw = tiles_w[w]
                    p_gcnT = pacc.tile([D, 128], F32, tag="p_gcnT")
                    p_sageT = pacc.tile([D, 128], F32, tag="p_sageT")
                    p_gath = []
                    for h in range(H1):
                        pg = pacc.tile([128, D + 1], F32, tag=f"p_gat{h}")
                        p_gath.append(pg)
                    for t in range(ntw):
                        Gt = gat.tile([128, WB], F32, tag="G")
                        nc.gpsimd.indirect_dma_start(
                            out=Gt[:], out_offset=None, in_=gathB[:],
                            in_offset=bass.IndirectOffsetOnAxis(
                                ap=S["idx_row"][:, t_glob:t_glob + 1], axis=0))
                        Gc = gat.tile([128, WB], F32, tag="Gc")
                        nc.vector.tensor_copy(Gc[:], Gt[:])
                        sbt = gat.tile([128, H1], F32, tag="sbt")
                        nc.gpsimd.indirect_dma_start(
                            out=sbt[:], out_offset=None, in_=adst[:],
                            in_offset=bass.IndirectOffsetOnAxis(
                                ap=S["idx_colL"][:, t_glob:t_glob + 1], axis=0))
                        zt = gat.tile([128, H1], F32, tag="z")
                        nc.vector.tensor_tensor(
                            out=zt[:], in0=Gc[:, D + 1:WB], in1=sbt[:],
                            op=ALU.add)
                        zs = gat.tile([128, H1], F32, tag="zs")
                        nc.vector.tensor_scalar(out=zs[:], in0=zt[:],
                                                scalar1=NEG_SLOPE, scalar2=None,
                                                op0=ALU.mult)
                        nc.vector.tensor_tensor(out=zt[:], in0=zt[:], in1=zs[:],
                                                op=ALU.max)
                        et = gat.tile([128, H1], F32, tag="E")
                        nc.scalar.activation(out=et[:], in_=zt[:], func=AF.Exp)

                        g64 = Gc[:, 0:D]
                        g65 = Gc[:, 0:D + 1]
                        cr = S["colrel"][:, t_glob:t_glob + 1]
                        st, sp = (t == 0), (t == ntw - 1)
                        Mg = mpool.tile([128, 128], F32, tag="Mg")
                        nc.vector.tensor_scalar(
                            out=Mg[:], in0=iota_f[:], scalar1=cr,
                            scalar2=S["wnorm"][:, t_glob:t_glob + 1],
                            op0=ALU.is_equal, op1=ALU.mult)
                        nc.tensor.matmul(out=p_gcnT[:], lhsT=g64, rhs=Mg[:],
                                         start=st, stop=sp)
                        Ms = mpool.tile([128, 128], F32, tag="Ms")
                        nc.vector.tensor_scalar(
                            out=Ms[:], in0=iota_f[:], scalar1=cr,
                            scalar2=S["wsage"][:, t_glob:t_glob + 1],
                            op0=ALU.is_equal, op1=ALU.mult)
                        nc.tensor.matmul(out=p_sageT[:], lhsT=g64, rhs=Ms[:],
                                         start=st, stop=sp)
                        for h in range(H1):
                            Mh = mpool.tile([128, 128], F32, tag="Mh")
                            nc.vector.tensor_scalar(
                                out=Mh[:], in0=iota_f[:], scalar1=cr,
                                scalar2=et[:, h:h + 1],
                                op0=ALU.is_equal, op1=ALU.mult)
                            nc.tensor.matmul(
                                out=p_gath[h][:], lhsT=Mh[:], rhs=g65,
                                start=st, stop=sp)
                        t_glob += 1

                    # ---------- window tails ----------
                    rows = slice(w * 128, (w + 1) * 128)
                    # GCN1: h1 = relu(s*(W1^T aggT) + b)
                    aggT = tl.tile([D, 128], F32, tag="aggT")
                    nc.vector.tensor_copy(aggT[:], p_gcnT[:])
                    ph1T = ptmp.tile([D, 128], F32, tag="pt")
                    nc.tensor.matmul(out=ph1T[:], lhsT=W["gcn_w1"][:],
                                     rhs=aggT[:])
                    h1Ts = tl.tile([D, 128], F32, tag="h1Ts")
                    nc.scalar.activation(out=h1Ts[:], in_=ph1T[:], func=AF.Relu,
                                         scale=W["gcn1_s"][:, :1],
                                         bias=W["gcn1_b"][:, :1])
                    h1Tv = tl.tile([D, 128], F32, tag="h1Tv")
                    nc.vector.tensor_copy(h1Tv[:], h1Ts[:])
                    ph1 = ptmp.tile([128, D], F32, tag="pt")
                    nc.tensor.matmul(out=ph1[:], lhsT=h1Tv[:], rhs=ident[:D, :D],
                                     is_transpose=True)
                    h1sb = tl.tile([128, D], F32, tag="h1sb")
                    nc.vector.tensor_copy(h1sb[:], ph1[:])
                    nc.sync.dma_start(out=bufC[rows, 0:D], in_=h1sb[:])

                    # GAT1 heads -> x2T halves -> h2, a2
                    x2TA = tl.tile([128, 128], F32, tag="x2TA")
                    x2TB = tl.tile([128, 128], F32, tag="x2TB")
                    for h in range(H1):
                        rd = tl.tile([128, 1], F32, tag="rd")
                        nc.vector.reciprocal(rd[:], p_gath[h][:, D:D + 1])
                        hd_sb = tl.tile([128, D], F32, tag="hd_sb")
                        nc.vector.tensor_scalar(
                            out=hd_sb[:], in0=p_gath[h][:, 0:D],
                            scalar1=rd[:, :1], scalar2=None, op0=ALU.mult)
                        pht = ptmp.tile([D, 128], F32, tag="pt")
                        nc.tensor.matmul(out=pht[:], lhsT=hd_sb[:], rhs=ident[:],
                                         is_transpose=True)
                        hdT = tl.tile([D, 128], F32, tag="hdT_g")
                        nc.vector.tensor_copy(hdT[:], pht[:])
                        pxh = ptmp.tile([D, 128], F32, tag="pt")
                        nc.tensor.matmul(out=pxh[:],
                                         lhsT=W["w1h"][:, h * D:(h + 1) * D],
                                         rhs=hdT[:])
                        stgt = x2TA if h < 2 else x2TB
                        nc.vector.tensor_copy(
                            stgt[(h % 2) * D:(h % 2 + 1) * D, :], pxh[:])
                    x2T = []
                    for half, px in enumerate((x2TA, x2TB)):
                        yT = tl.tile([128, 128], F32, tag="yT")
                        nc.vector.tensor_scalar(
                            out=yT[:], in0=px[:],
                            scalar1=W["b1c"][:, half:half + 1], scalar2=None,
                            op0=ALU.add)
                        ymin = tl.tile([128, 128], F32, tag="ymin")
                        nc.vector.tensor_scalar(out=ymin[:], in0=yT[:],
                                                scalar1=0.0, scalar2=None,
                                                op0=ALU.min)
                        yexp = tl.tile([128, 128], F32, tag="yexp")
                        nc.scalar.activation(out=yexp[:], in_=ymin[:],
                                             func=AF.Exp)
                        ye1 = tl.tile([128, 128], F32, tag="ye1")
                        nc.vector.tensor_scalar(out=ye1[:], in0=yexp[:],
                                                scalar1=-1.0, scalar2=None,
                                                op0=ALU.add)
                        ymax = tl.tile([128, 128], F32, tag="ymax")
                        nc.vector.tensor_scalar(out=ymax[:], in0=yT[:],
                                                scalar1=0.0, scalar2=None,
                                                op0=ALU.max)
                        xt2 = tl.tile([128, 128], F32, tag=f"x2T{half}")
                        nc.vector.tensor_tensor(out=xt2[:], in0=ymax[:],
                                                in1=ye1[:], op=ALU.add)
                        x2T.append(xt2)
                    ph2T = ptmp.tile([D, 128], F32, tag="pt")
                    nc.tensor.matmul(out=ph2T[:], lhsT=W["w2A"][:],
                                     rhs=x2T[0][:], start=True, stop=False)
                    nc.tensor.matmul(out=ph2T[:], lhsT=W["w2B"][:],
                                     rhs=x2T[1][:], start=False, stop=True)
                    pa2T = ptmp.tile([2, 128], F32, tag="pt")
                    nc.tensor.matmul(out=pa2T[:], lhsT=W["v2u2"][:, 0:2],
                                     rhs=x2T[0][:], start=True, stop=False)
                    nc.tensor.matmul(out=pa2T[:], lhsT=W["v2u2"][:, 2:4],
                                     rhs=x2T[1][:], start=False, stop=True)
                    h2Ts = tl.tile([D, 128], F32, tag="h2Ts")
                    nc.vector.tensor_copy(h2Ts[:], ph2T[:])
                    ph2 = ptmp.tile([128, D], F32, tag="pt")
                    nc.tensor.matmul(out=ph2[:], lhsT=h2Ts[:], rhs=ident[:D, :D],
                                     is_transpose=True)
                    h2sb = tl.tile([128, D], F32, tag="h2sb")
                    nc.vector.tensor_copy(h2sb[:], ph2[:])
                    nc.sync.dma_start(out=bufC[rows, D:2 * D], in_=h2sb[:])
                    nc.sync.dma_start(out=bufC[rows, 2 * D:2 * D + 1],
                                      in_=ones_col[:])
                    a2Ts = tl.tile([2, 128], F32, tag="a2Ts")
                    nc.vector.tensor_copy(a2Ts[:], pa2T[:])
                    pa2 = ptmp.tile([128, 2], F32, tag="pt")
                    nc.tensor.matmul(out=pa2[:], lhsT=a2Ts[:], rhs=ident[:2, :2],
                                     is_transpose=True)
                    a2sb = tl.tile([128, 2], F32, tag="a2sb")
                    nc.vector.tensor_copy(a2sb[:], pa2[:])
                    nc.sync.dma_start(out=bufC[rows, WC - 1:WC],
                                      in_=a2sb[:, 0:1])
                    nc.sync.dma_start(out=a2dst[rows, :], in_=a2sb[:, 1:2])

                    # SAGE1
                    meanT = tl.tile([D, 128], F32, tag="meanT")
                    nc.vector.tensor_copy(meanT[:], p_sageT[:])
                    xd0 = tl.tile([128, D], F16, tag="xd0")
                    nc.sync.dma_start(out=xd0[:], in_=dr["xs65"][rows, 0:D])
                    xd = tl.tile([128, D], F32, tag="xd")
                    nc.vector.tensor_copy(xd[:], xd0[:])
                    pxdT = ptmp.tile([D, 128], F32, tag="pt")
                    nc.tensor.matmul(out=pxdT[:], lhsT=xd[:], rhs=ident[:],
                                     is_transpose=True)
                    xdT = tl.tile([D, 128], F32, tag="xdT")
                    nc.vector.tensor_copy(xdT[:], pxdT[:])
                    psT = ptmp.tile([D, 128], F32, tag="pt")
                    nc.tensor.matmul(out=psT[:], lhsT=W["sage_wl1"][:],
                                     rhs=meanT[:], start=True, stop=False)
                    nc.tensor.matmul(out=psT[:], lhsT=W["sage_wr1"][:],
                                     rhs=xdT[:], start=False, stop=True)
                    sTs = tl.tile([D, 128], F32, tag="sTs")
                    nc.scalar.activation(out=sTs[:], in_=psT[:],
                                         func=AF.Identity,
                                         bias=W["sage_bl1"][:, :1])
                    sTv = tl.tile([D, 128], F32, tag="sTv")
                    nc.vector.tensor_copy(sTv[:], sTs[:])
                    ps_ = ptmp.tile([128, D], F32, tag="pt")
                    nc.tensor.matmul(out=ps_[:], lhsT=sTv[:], rhs=ident[:D, :D],
                                     is_transpose=True)
                    s_sb = tl.tile([128, D], F32, tag="s_sb")
                    nc.vector.tensor_copy(s_sb[:], ps_[:])
                    sq = tl.tile([128, D], F32, tag="sq")
                    nc.vector.tensor_tensor(out=sq[:], in0=s_sb[:], in1=s_sb[:],
                                            op=ALU.mult)
                    ssum = tl.tile([128, 1], F32, tag="ssum")
                    nc.vector.tensor_reduce(out=ssum[:], in_=sq[:],
                                            axis=mybir.AxisListType.X,
                                            op=ALU.add)
                    nc.vector.tensor_scalar(out=ssum[:], in0=ssum[:],
                                            scalar1=1e-24, scalar2=None,
                                            op0=ALU.add)
                    rs = tl.tile([128, 1], F32, tag="rs")
                    nc.vector.reciprocal(rs[:], ssum[:])
                    rq = tl.tile([128, 1], F32, tag="rq")
                    nc.scalar.activation(out=rq[:], in_=rs[:], func=AF.Sqrt)
                    nc.vector.tensor_scalar(out=st_hs[:, w * D:(w + 1) * D],
                                            in0=s_sb[:], scalar1=rq[:, :1],
                                            scalar2=0.0, op0=ALU.mult,
                                            op1=ALU.max)
                    nc.sync.dma_start(out=bufC[rows, 2 * D + 1:3 * D + 1],
                                      in_=st_hs[:, w * D:(w + 1) * D])

            # ============================================== AllGather #2
            nc.gpsimd.collective_compute(
                "AllGather", ALU.bypass,
                replica_groups=[list(range(NC_N))],
                ins=[bufC[:].opt()], outs=[gathC[:].opt()])

            # ================================================== phase C
            with (
                tc.tile_pool(name="gatC", bufs=8) as gat,
                tc.tile_pool(name="mC", bufs=8) as mpool,
                tc.tile_pool(name="tlC", bufs=4) as tl,
                tc.tile_pool(name="paccC", bufs=1, space="PSUM") as pacc,
                tc.tile_pool(name="ptmpC", bufs=2, space="PSUM") as ptmp,
            ):
                t_glob = 0
                for w in range(nw):
                    ntw = tiles_w[w]
                    p_g2T = pacc.tile([D, 128], F32, tag="p_g2T")
                    p_s2T = pacc.tile([D, 128], F32, tag="p_s2T")
                    p_gat2 = pacc.tile([128, D + 1], F32, tag="p_gat2")
                    for t in range(ntw):
                        Gt = gat.tile([128, WC], F32, tag="G2")
                        nc.gpsimd.indirect_dma_start(
                            out=Gt[:], out_offset=None, in_=gathC[:],
                            in_offset=bass.IndirectOffsetOnAxis(
                                ap=S["idx_row"][:, t_glob:t_glob + 1], axis=0))
                        Gc = gat.tile([128, WC], F32, tag="Gc2")
                        nc.vector.tensor_copy(Gc[:], Gt[:])
                        sbt = gat.tile([128, 1], F32, tag="sb2")
                        nc.gpsimd.indirect_dma_start(
                            out=sbt[:], out_offset=None, in_=a2dst[:],
                            in_offset=bass.IndirectOffsetOnAxis(
                                ap=S["idx_colL"][:, t_glob:t_glob + 1], axis=0))
                        z2 = gat.tile([128, 1], F32, tag="z2")
                        nc.vector.tensor_tensor(
                            out=z2[:], in0=Gc[:, WC - 1:WC], in1=sbt[:],
                            op=ALU.add)
                        z2s = gat.tile([128, 1], F32, tag="z2s")
                        nc.vector.tensor_scalar(out=z2s[:], in0=z2[:],
                                                scalar1=NEG_SLOPE, scalar2=None,
                                                op0=ALU.mult)
                        nc.vector.tensor_tensor(out=z2[:], in0=z2[:], in1=z2s[:],
                                                op=ALU.max)
                        e2 = gat.tile([128, 1], F32, tag="E2")
                        nc.scalar.activation(out=e2[:], in_=z2[:], func=AF.Exp)

                        g1s = Gc[:, 0:D]
                        g2s = Gc[:, D:2 * D + 1]
                        g3s = Gc[:, 2 * D + 1:3 * D + 1]
                        cr = S["colrel"][:, t_glob:t_glob + 1]
                        st, sp = (t == 0), (t == ntw - 1)
                        Mg = mpool.tile([128, 128], F32, tag="Mg")
                        nc.vector.tensor_scalar(
                            out=Mg[:], in0=iota_f[:], scalar1=cr,
                            scalar2=S["wnorm"][:, t_glob:t_glob + 1],
                            op0=ALU.is_equal, op1=ALU.mult)
                        nc.tensor.matmul(out=p_g2T[:], lhsT=g1s, rhs=Mg[:],
                                         start=st, stop=sp)
                        Ms = mpool.tile([128, 128], F32, tag="Ms")
                        nc.vector.tensor_scalar(
                            out=Ms[:], in0=iota_f[:], scalar1=cr,
                            scalar2=S["wsage"][:, t_glob:t_glob + 1],
                            op0=ALU.is_equal, op1=ALU.mult)
                        nc.tensor.matmul(out=p_s2T[:], lhsT=g3s, rhs=Ms[:],
                                         start=st, stop=sp)
                        Mh = mpool.tile([128, 128], F32, tag="Mh")
                        nc.vector.tensor_scalar(
                            out=Mh[:], in0=iota_f[:], scalar1=cr,
                            scalar2=e2[:, 0:1],
                            op0=ALU.is_equal, op1=ALU.mult)
                        nc.tensor.matmul(out=p_gat2[:], lhsT=Mh[:], rhs=g2s,
                                         start=st, stop=sp)
                        t_glob += 1

                    # ---------- window tails ----------
                    # GCN2 (+w0, +w0*b2)
                    aggT = tl.tile([D, 128], F32, tag="aggT")
                    nc.vector.tensor_copy(aggT[:], p_g2T[:])
                    poT = ptmp.tile([D, 128], F32, tag="pt")
                    nc.tensor.matmul(out=poT[:], lhsT=W["gcn_w2"][:],
                                     rhs=aggT[:])
                    oTs = tl.tile([D, 128], F32, tag="oTs")
                    nc.scalar.activation(out=oTs[:], in_=poT[:],
                                         func=AF.Identity,
                                         scale=w64[:, 0:1], bias=b2w0[:, :1])
                    oTv = tl.tile([D, 128], F32, tag="oTv")
                    nc.vector.tensor_copy(oTv[:], oTs[:])
                    po = ptmp.tile([128, D], F32, tag="pt")
                    nc.tensor.matmul(out=po[:], lhsT=oTv[:], rhs=ident[:D, :D],
                                     is_transpose=True)
                    ogcn = tl.tile([128, D], F32, tag="ogcn")
                    nc.vector.tensor_copy(ogcn[:], po[:])

                    # GAT2 (+w1)
                    rd = tl.tile([128, 1], F32, tag="rd")
                    nc.vector.reciprocal(rd[:], p_gat2[:, D:D + 1])
                    ogat = tl.tile([128, D], F32, tag="ogat")
                    nc.vector.tensor_scalar(out=ogat[:], in0=p_gat2[:, 0:D],
                                            scalar1=rd[:, :1],
                                            scalar2=wc[:, 1:2],
                                            op0=ALU.mult, op1=ALU.mult)

                    # SAGE2 (+w2); root rows come from the SBUF staging
                    meanT = tl.tile([D, 128], F32, tag="meanT")
                    nc.vector.tensor_copy(meanT[:], p_s2T[:])
                    phdT = ptmp.tile([D, 128], F32, tag="pt")
                    nc.tensor.matmul(out=phdT[:],
                                     lhsT=st_hs[:, w * D:(w + 1) * D],
                                     rhs=ident[:], is_transpose=True)
                    hdT = tl.tile([D, 128], F32, tag="hdT")
                    nc.vector.tensor_copy(hdT[:], phdT[:])
                    psT = ptmp.tile([D, 128], F32, tag="pt")
                    nc.tensor.matmul(out=psT[:], lhsT=W["sage_wl2"][:],
                                     rhs=meanT[:], start=True, stop=False)
                    nc.tensor.matmul(out=psT[:], lhsT=W["sage_wr2"][:],
                                     rhs=hdT[:], start=False, stop=True)
                    sTs = tl.tile([D, 128], F32, tag="sTs")
                    nc.scalar.activation(out=sTs[:], in_=psT[:],
                                         func=AF.Identity,
                                         bias=W["sage_bl2c"][:, :1])
                    sTv = tl.tile([D, 128], F32, tag="sTv")
                    nc.vector.tensor_copy(sTv[:], sTs[:])
                    ps_ = ptmp.tile([128, D], F32, tag="pt")
                    nc.tensor.matmul(out=ps_[:], lhsT=sTv[:], rhs=ident[:D, :D],
                                     is_transpose=True)
                    s_sb = tl.tile([128, D], F32, tag="s_sb")
                    nc.vector.tensor_copy(s_sb[:], ps_[:])
                    sq = tl.tile([128, D], F32, tag="sq")
                    nc.vector.tensor_tensor(out=sq[:], in0=s_sb[:], in1=s_sb[:],
                                            op=ALU.mult)
                    ssum = tl.tile([128, 1], F32, tag="ssum")
                    nc.vector.tensor_reduce(out=ssum[:], in_=sq[:],
                                            axis=mybir.AxisListType.X,
                                            op=ALU.add)
                    nc.vector.tensor_scalar(out=ssum[:], in0=ssum[:],
                                            scalar1=1e-24, scalar2=None,
                                            op0=ALU.add)
                    rs = tl.tile([128, 1], F32, tag="rs")
                    nc.vector.reciprocal(rs[:], ssum[:])
                    rq = tl.tile([128, 1], F32, tag="rq")
                    nc.scalar.activation(out=rq[:], in_=rs[:], func=AF.Sqrt)
                    osage = tl.tile([128, D], F32, tag="osage")
                    nc.vector.tensor_scalar(out=osage[:], in0=s_sb[:],
                                            scalar1=rq[:, :1],
                                            scalar2=wc[:, 2:3],
                                            op0=ALU.mult, op1=ALU.mult)

                    # mix
                    mx1 = tl.tile([128, D], F32, tag="mx1")
                    nc.vector.tensor_tensor(out=mx1[:], in0=ogcn[:],
                                            in1=ogat[:], op=ALU.add)
                    mx2 = tl.tile([128, D], F32, tag="mx2")
                    nc.vector.tensor_tensor(out=mx2[:], in0=mx1[:],
                                            in1=osage[:], op=ALU.add)
                    nc.vector.tensor_tensor(out=st_out[:, w * D:(w + 1) * D],
                                            in0=mx2[:], in1=bgat[:],
                                            op=ALU.add)

            # ---- final output DMA: full windows, then the partial tail
            out_ap = bass.AP(out, 0, [[D, 128], [128 * D, nw - 1], [1, D]])
            nc.sync.dma_start(
                out=out_ap,
                in_=st_out[:, 0:(nw - 1) * D].rearrange(
                    "p (w c) -> p w c", w=nw - 1))
            nc.sync.dma_start(
                out=out[(nw - 1) * 128:shard, :],
                in_=st_out[0:rem, (nw - 1) * D:nw * D])
    return nc


# ---------------------------------------------------------------- host logic
DEBUG = {}
_PROG_CACHE = {}


def _run(nc, in_maps, trace=False):
    import time as _time
    if not nc.is_finalized():
        nc.finalize()
    t0 = _time.perf_counter()
    res = run_bass_kernel_spmd(nc, in_maps, list(range(NC_N)), trace=trace)
    DEBUG.setdefault("run_walls", []).append(_time.perf_counter() - t0)
    if res.exec_time_ns:
        DEBUG.setdefault("exec_ns", []).append(res.exec_time_ns)
    return res.results


def gnn_forward(x, edge_index, gate_w1, gate_b1, gate_w2, gate_b2,
                gcn_w1, gcn_b1, bn_gamma, bn_beta, gcn_w2, gcn_b2,
                gat_w1, gat_att_src1, gat_att_dst1, gat_b1,
                gat_w2, gat_att_src2, gat_att_dst2, gat_b2,
                sage_wl1, sage_bl1, sage_wr1, sage_wl2, sage_bl2, sage_wr2,
                trace=False):
    n_nodes = x.shape[0]
    x = np.asarray(x, np.float32)
    streams, tiles_w, T, shard, nw = build_schedule(
        np.asarray(edge_index), n_nodes)
    npad = nw * 128

    # ---- host weight folding (weights only, no data)
    w1r = np.asarray(gat_w1, np.float32).reshape(D, H1, D)
    vsrc = np.einsum("chj,hj->ch", w1r, np.asarray(gat_att_src1, np.float32))
    vdst = np.einsum("chj,hj->ch", w1r, np.asarray(gat_att_dst1, np.float32))
    vcat = np.concatenate([vsrc, vdst], axis=1).astype(np.float32)  # [64,8]
    v2 = (np.asarray(gat_w2, np.float32) @
          np.asarray(gat_att_src2, np.float32)[0])  # [256]
    u2 = (np.asarray(gat_w2, np.float32) @
          np.asarray(gat_att_dst2, np.float32)[0])
    v2u2 = np.stack([v2[:128], u2[:128], v2[128:], u2[128:]],
                    axis=1).astype(np.float32)  # [128,4]
    bn_s = (np.asarray(bn_gamma, np.float32) /
            np.sqrt(np.float32(1.0 + BN_EPS)))
    gcn1_s = bn_s.reshape(D, 1).astype(np.float32)
    gcn1_b = (bn_s * np.asarray(gcn_b1, np.float32) +
              np.asarray(bn_beta, np.float32)).reshape(D, 1).astype(np.float32)

    ck = (n_nodes, T, tuple(tiles_w))
    if ck in _PROG_CACHE:
        nc = _PROG_CACHE[ck]
    else:
        nc = build_full(n_nodes, tiles_w, T)
        _PROG_CACHE[ck] = nc

    common = {
        "vcat": vcat,
        "gw1": np.asarray(gate_w1, np.float32),
        "gb1": np.asarray(gate_b1, np.float32).reshape(1, D),
        "gw2": np.asarray(gate_w2, np.float32),
        "gb2": np.asarray(gate_b2, np.float32).reshape(1, 3),
        "gcn_w1": np.asarray(gcn_w1, np.float32),
        "gcn1_s": gcn1_s, "gcn1_b": gcn1_b,
        "sage_wl1": np.asarray(sage_wl1, np.float32),
        "sage_wr1": np.asarray(sage_wr1, np.float32),
        "sage_bl1": np.asarray(sage_bl1, np.float32).reshape(D, 1),
        "w2A": np.asarray(gat_w2, np.float32)[:128],
        "w2B": np.asarray(gat_w2, np.float32)[128:],
        "v2u2": v2u2,
        "w1h": np.asarray(gat_w1, np.float32),
        "b1c": np.asarray(gat_b1, np.float32).reshape(2, 128).T.copy(),
        "gcn_w2": np.asarray(gcn_w2, np.float32),
        "gcn_b2c": np.asarray(gcn_b2, np.float32).reshape(D, 1),
        "sage_wl2": np.asarray(sage_wl2, np.float32),
        "sage_wr2": np.asarray(sage_wr2, np.float32),
        "sage_bl2c": np.asarray(sage_bl2, np.float32).reshape(D, 1),
        "gat_b2r": np.asarray(gat_b2, np.float32).reshape(1, D),
    }
    in_maps = []
    for k in range(NC_N):
        m = dict(common)
        m.update(streams[k])
        xs65 = np.zeros((npad, D + 1), np.float16)
        xs65[:shard, :D] = x[k * shard:(k + 1) * shard]
        xs65[:shard, D] = 1.0
        m["xs65"] = xs65
        in_maps.append(m)
    res = _run(nc, in_maps, trace=trace)
    out = np.concatenate([res[k]["out"] for k in range(NC_N)], 0)
    return out.astype(np.float32)


def kernel(**inputs):
    return gnn_forward(**inputs)
